# revision 18
# baseline (speedup 1.0000x reference)
# kernel.py — Trainium2 Bass kernel for nn_ChannelAttentionBlock (v3)
#
# Computation (per reference):
#   h = relu(feature @ fc1_w.T + fc1_b)            [B,C,FF]
#   f = h @ fc2_w.T + fc2_b                        [B,C,HW]
#   T[b,n,m] = sum_c x[b,c,n] * f[b,c,m] * ls[m]   (ls = exp(min(logit_scale, log 100)))
#   P = softmax_n(T);  out[b,n,c] = sum_m P[n,m] x[b,c,m];  LayerNorm over c; -> [B,C,HW]
#
# Sharding (8 cores):
#   MLP tensor-parallel: fc1 split on hidden (each core 1152 of 9216 hidden),
#   fc2 split on input; ReduceScatter over batch so core k receives f[b=k].
#   Attention data-parallel: core k handles batch k entirely.
#
# Precision (PE streams 1 col/cyc for bf16/fp16, fp32r is 1 cyc when N>=256):
#   MLP1/MLP2: fp16 weights single-plane + fp16 hi/lo activations, 2 pairings
#     (fh.w + fl.w) — measured 7e-3 rel err vs the 2e-2 budget. Halves w1 DMA
#     (21.2MB/core) and cuts MLP PE work by a third vs bf16 3-way.
#   mm1 (T = f^T x, K=24): bf16 hi/lo K-stacked — full precision, free in K.
#   mm2 (out += xp^T E): fp32r on fp32 et (1cyc/row at N=512).
#   Softmax shift: per-m Cauchy-Schwarz bound -ls*xmax*||f[:,m]||+40.
#
# Schedule (v3): phase-separated for PE clock-gate hygiene. The PE drops to
# 1.2 GHz after a >3.4us idle and only re-promotes after ~3us of gapless
# execution, so work is organized in long wait-free streaks:
#   [w1 DMA || MLP1] -> [MLP2 all 4 groups back-to-back; RS(g) issued as each
#   group finishes, RS wait hidden under later groups] -> [attention: 32
#   m-block slots; 2048-wide exp on 2x[128,4,512] PSUM rotation (all 8 banks);
#   mm2 of the lagged pair writes into exp-drained sub-banks of the previous
#   tile; per-slot PE burst ~3.5us vs ACT 4.2us] -> LayerNorm.

import os
import numpy as np

B, C, HW, FF, P = 8, 6, 4096, 9216, 128
NCORES = 8
HS = FF // NCORES        # 1152
KT1 = FF // P            # 72 fc1 K tiles
CH1 = 3                  # fc1 K tiles per DMA chunk
NC1 = KT1 // CH1         # 24 fc1 chunks
JT = HS // P             # 9  fc2 K tiles
NT = HW // 512           # 8  512-wide n/o chunks
MBS = HW // P            # 32 m blocks
NGRP = 4                 # reduce-scatter groups (2 o-chunks each)
MB_PER_G = MBS // NGRP   # 8
EPS = 1e-5
SHIFT_HEADROOM = 40.0

_cache = {}


def _build_program():
    import concourse.bacc as bacc
    import concourse.bass as bass
    import concourse.tile as tile
    import concourse.mybir as mybir

    dt = mybir.dt.float32
    dtr = mybir.dt.float32r
    dtb = mybir.dt.bfloat16
    dth = mybir.dt.float16
    AF = mybir.ActivationFunctionType
    ALU = mybir.AluOpType
    AX = mybir.AxisListType

    nc = bacc.Bacc(
        "TRN2",
        target_bir_lowering=False,
        debug=False,
        enable_asserts=False,
        num_devices=NCORES,
    )

    # ---- external I/O ----
    featT_d = nc.dram_tensor("featT", [P, KT1, 2, 48], dth, kind="ExternalInput").ap()
    w1_d = nc.dram_tensor("w1t", [NC1, P, CH1, HS], dth, kind="ExternalInput").ap()
    b1_d = nc.dram_tensor("b1", [1, HS], dt, kind="ExternalInput").ap()
    w2_d = nc.dram_tensor("w2t", [NGRP, P, JT, 1024], dth, kind="ExternalInput").ap()
    b2_d = nc.dram_tensor("b2", [1, HW], dt, kind="ExternalInput").ap()  # fc2_b/8
    xaug_d = nc.dram_tensor("xaug", [24, HW], dtb, kind="ExternalInput").ap()
    xtb_d = nc.dram_tensor("xtb", [P, MBS, C], dt, kind="ExternalInput").ap()
    ls_d = nc.dram_tensor("lsb", [P, MBS], dt, kind="ExternalInput").ap()
    srt_d = nc.dram_tensor("srtc", [P, 2], mybir.dt.int32,
                           kind="ExternalInput").ap()
    ones_d = nc.dram_tensor("ones1", [1, 48], dt, kind="ExternalInput").ap()
    id48_d = nc.dram_tensor("id48", [48, 48], dt, kind="ExternalInput").ap()
    blk_d = nc.dram_tensor("blk", [48, NT], dt, kind="ExternalInput").ap()
    blkT_d = nc.dram_tensor("blkT", [NT, 48], dt, kind="ExternalInput").ap()
    wb48_d = nc.dram_tensor("wb48", [48, 2], dt, kind="ExternalInput").ap()
    negx_d = nc.dram_tensor("negx", [P, 1], dt, kind="ExternalInput").ap()
    out_d = nc.dram_tensor("out", [C, HW], dt, kind="ExternalOutput").ap()

    with tile.TileContext(nc) as tc:
        # float32r APs carry full-fp32 bit patterns; the PE rounds at load.
        with nc.allow_low_precision(reason="fp32r/fp16/bf16 kernel dataflow"), \
             tc.tile_pool(name="const", bufs=1) as const, \
             tc.tile_pool(name="dram", bufs=1, space="DRAM") as dram:

            # ---- constants / small inputs ----
            xaug_sb = const.tile([24, HW], dtb, tag="xaug")
            nc.gpsimd.dma_start(out=xaug_sb[:], in_=xaug_d)
            xtb_sb = const.tile([P, MBS, C], dt, tag="xtb")
            nc.gpsimd.dma_start(out=xtb_sb[:], in_=xtb_d)
            ls_sb = const.tile([P, MBS], dt, tag="ls")
            nc.gpsimd.dma_start(out=ls_sb[:], in_=ls_d)
            ones_sb = const.tile([1, 48], dtr, tag="ones1")
            nc.gpsimd.dma_start(out=ones_sb[:], in_=ones_d.bitcast(dtr))
            id48_sb = const.tile([48, 48], dt, tag="id48")
            nc.gpsimd.dma_start(out=id48_sb[:], in_=id48_d)
            negx_sb = const.tile([P, 1], dt, tag="negx")
            nc.gpsimd.dma_start(out=negx_sb[:], in_=negx_d)
            srt_sb = const.tile([P, 2], mybir.dt.int32, tag="srtc")
            nc.gpsimd.dma_start(out=srt_sb[:], in_=srt_d)
            blk_sb = const.tile([48, NT], dtr, tag="blk")
            nc.gpsimd.dma_start(out=blk_sb[:], in_=blk_d.bitcast(dtr))
            blkT_sb = const.tile([NT, 48], dtr, tag="blkT")
            nc.gpsimd.dma_start(out=blkT_sb[:], in_=blkT_d.bitcast(dtr))
            wb48_sb = const.tile([48, 2], dt, tag="wb48")
            nc.gpsimd.dma_start(out=wb48_sb[:], in_=wb48_d)

            # h^T in fp16 hi/lo planes (filled after MLP1)
            hT_sb = const.tile([P, JT, 2, 48], dth, tag="hT")
            faug_sb = const.tile([24, HW], dtb, tag="faug")
            # per-m softmax shift (exp bias): -ls*xmax*||f[:,m]|| + 40
            shift_sb = const.tile([P, MBS], dt, tag="shift")
            zero128_sb = const.tile([P, 1], dt, tag="zero128")
            nc.vector.memset(zero128_sb[:], 0.0)
            ones6_sb = const.tile([C, 1], dtb, tag="ones6")
            nc.vector.memset(ones6_sb[:], 1.0)
            # attention output accumulators, one [6,512] tile per n-chunk
            O_nt = [const.tile([C, 512], dt, tag=f"O{nt}", name=f"O{nt}")
                    for nt in range(NT)]
            wz_sb = const.tile([1, 640], dt, tag="wz")
            nc.vector.memset(wz_sb[:], 0.0)
            O48_sb = const.tile([48, 512], dt, tag="O48")

            # DRAM bounce buffers for the reduce-scatter groups
            rs_in = [dram.tile([48, 2, 512], dt, tag=f"rsin{g}", name=f"rsin{g}")
                     for g in range(NGRP)]
            rs_out = [dram.tile([C, 2, 512], dt, tag=f"rsout{g}", name=f"rsout{g}")
                      for g in range(NGRP)]
            wrm_in = dram.tile([1, 8], dt, tag="wrmin", name="wrmin")
            wrm_out = dram.tile([1, 8], dt, tag="wrmout", name="wrmout")

            # ============ warmup: PE clock promote, exp tables, CC cores ============
            with tc.tile_pool(name="wrm", bufs=1, space="PSUM") as wrmp:
                wps = wrmp.tile([P, 512], dt, tag="wps")
                for _ in range(12):
                    nc.tensor.matmul(
                        wps[:],
                        lhsT=wz_sb[:, 0:P].bitcast(dtr),
                        rhs=wz_sb[:, P:P + 512].bitcast(dtr),
                        start=True,
                        stop=True,
                    )
                # load the exp table set now (Exp+Relu share it; the kernel
                # body uses no other table until the final LayerNorm)
                wex = wrmp.tile([1, 8], dt, tag="wex")
                nc.scalar.activation(wex[:], wz_sb[:, 0:8], AF.Exp,
                                     bias=zero128_sb[0:1, :])
            # init + warm the collective path (RDH state, cross-core skew)
            nc.gpsimd.dma_start(out=wrm_in[:], in_=wz_sb[:, 0:8])
            nc.gpsimd.collective_compute(
                "AllReduce",
                ALU.add,
                replica_groups=[list(range(NCORES))],
                ins=[wrm_in.opt()],
                outs=[wrm_out.opt()],
            )

            # ====== MLP1 (w2 DMAs interleaved) / MLP2 + RS / fprep(0) ======
            with tc.tile_pool(name="w2p", bufs=NGRP) as w2p, \
                 tc.tile_pool(name="m2c", bufs=1) as m2c, \
                 tc.tile_pool(name="fts", bufs=2) as ftsp, \
                 tc.tile_pool(name="fps", bufs=2, space="PSUM") as fps, \
                 tc.tile_pool(name="fpe", bufs=2) as fpep:

                def emit_fprep(g, get_ptile):
                    # runs once RS(g) lands: distribute f, build faug + shift.
                    # get_ptile() -> [P, 512] PSUM AP for the norm matmuls.
                    gsl = slice(g * 1024, (g + 1) * 1024)
                    fgrp = ftsp.tile([C, 1024], dt, tag="fgrp")
                    nc.sync.dma_start(out=fgrp[:], in_=rs_out[g][:])
                    # bf16 hi/lo split of f for mm1, rows [fh,fh,fl,fl]
                    fh = ftsp.tile([C, 1024], dtb, tag="fh")
                    nc.vector.tensor_copy(fh[:], fgrp[:])
                    fl = ftsp.tile([C, 1024], dtb, tag="fl")
                    nc.vector.tensor_sub(fl[:], fgrp[:], fh[:])
                    nc.sync.dma_start(out=faug_sb[0:C, gsl], in_=fh[:])
                    nc.sync.dma_start(out=faug_sb[C:2 * C, gsl], in_=fh[:])
                    nc.sync.dma_start(out=faug_sb[2 * C:3 * C, gsl], in_=fl[:])
                    nc.sync.dma_start(out=faug_sb[3 * C:24, gsl], in_=fl[:])
                    # shift[m] = -ls[m]*xmax*||f[:,m]||_2 + 40; column sq-norms
                    # via tiny matmuls into a borrowed PSUM tile, then sqrt on
                    # DVE via the bitcast seed (+-3.5%, harmless in a bound
                    # with ~47 e-folds of margin). No ACT => no table reloads.
                    fsq = ftsp.tile([C, 1024], dtb, tag="fsq")
                    nc.vector.tensor_mul(fsq[:], fgrp[:], fgrp[:])
                    pt = get_ptile()
                    for chn in range(NT):
                        nc.tensor.matmul(
                            pt[:, chn:chn + 1],
                            lhsT=fsq[:, chn * P:(chn + 1) * P],
                            rhs=ones6_sb[:],
                            start=True,
                            stop=True,
                        )
                    # y = bitcast(i >> 1) = sqrt(S) * 2^-63.5 * (1..1.061);
                    # the 2^63.5 rescale is folded into negx on the host
                    fns = ftsp.tile([P, NT], mybir.dt.int32, tag="fns")
                    nc.vector.tensor_scalar(
                        out=fns[:], in0=pt[:, 0:NT].bitcast(mybir.dt.int32),
                        scalar1=srt_sb[:, 0:1], scalar2=0.0,
                        op0=ALU.arith_shift_right, op1=ALU.bypass,
                    )
                    fnl = ftsp.tile([P, NT], dt, tag="fnl")
                    nc.vector.tensor_mul(
                        fnl[:], fns[:].bitcast(dt),
                        ls_sb[:, NT * g:NT * (g + 1)])
                    nc.vector.tensor_scalar(
                        out=shift_sb[:, NT * g:NT * (g + 1)], in0=fnl[:],
                        scalar1=negx_sb[:], scalar2=SHIFT_HEADROOM,
                        op0=ALU.mult, op1=ALU.add,
                    )

                b2_sb = m2c.tile([1, HW], dtr, tag="b2")
                nc.gpsimd.dma_start(out=b2_sb[:], in_=b2_d.bitcast(dtr))
                w2t = {}

                def fetch_w2(g):
                    w2t[g] = w2p.tile([P, JT, 1024], dth, tag="w2t",
                                      name=f"w2t{g}")
                    nc.sync.dma_start(out=w2t[g][:], in_=w2_d[g])

                w2_dma_at = {10: 0, 16: 1}

                # ---- MLP1: h = relu(feat @ w1 + b1), fp16 2-way ----
                with tc.tile_pool(name="w1p", bufs=4) as w1p, \
                     tc.tile_pool(name="m1c", bufs=1) as m1c, \
                     tc.tile_pool(name="ps1", bufs=1, space="PSUM") as ps1, \
                     tc.tile_pool(name="pst", bufs=2, space="PSUM") as pst:
                    featT_sb = m1c.tile([P, KT1, 2, 48], dth, tag="featT")
                    nc.sync.dma_start(out=featT_sb[:], in_=featT_d)
                    b1_sb = m1c.tile([1, HS], dtr, tag="b1")
                    nc.gpsimd.dma_start(out=b1_sb[:], in_=b1_d.bitcast(dtr))
                    h_sb = m1c.tile([48, HS], dt, tag="h")
                    hp = ps1.tile([48, 3, 512], dt, tag="hp")
                    for ch in range(NC1):
                        w1t = w1p.tile([P, CH1, HS], dth, tag="w1t")
                        nc.sync.dma_start(out=w1t[:], in_=w1_d[ch])
                        if ch in w2_dma_at:  # prefetch w2 in MLP1's DMA shadow
                            fetch_w2(w2_dma_at[ch])
                        for s in range(CH1):
                            kk = CH1 * ch + s
                            for hl in range(2):
                                lhs = featT_sb[:, kk, hl, :]
                                for j in range(3):
                                    nc.tensor.matmul(
                                        hp[:, j, 0:384],
                                        lhsT=lhs,
                                        rhs=w1t[:, s, j * 384:(j + 1) * 384],
                                        start=(kk == 0 and hl == 0),
                                        stop=False,
                                    )
                    for j in range(3):  # bias via K=1 ones row (fp32r)
                        nc.tensor.matmul(
                            hp[:, j, 0:384],
                            lhsT=ones_sb[:],
                            rhs=b1_sb[:, j * 384:(j + 1) * 384],
                            start=False,
                            stop=True,
                        )
                    for j in range(3):  # relu eviction on ACT
                        nc.scalar.activation(
                            h_sb[:, j * 384:(j + 1) * 384],
                            hp[:, j, 0:384],
                            AF.Relu,
                        )
                    # transpose h -> hT (9 PE transposes), split fp16 hi/lo
                    for t in range(JT):
                        tp = pst.tile([P, 48], dt, tag="tp")
                        nc.tensor.transpose(
                            tp[:], h_sb[:, t * P:(t + 1) * P], id48_sb[:]
                        )
                        nc.vector.tensor_copy(hT_sb[:, t, 0, :], tp[:])
                        nc.vector.tensor_sub(hT_sb[:, t, 1, :], tp[:],
                                             hT_sb[:, t, 0, :])

                # ---- MLP2 groups + RS; fprep(0) tucked after group 1 so its
                # faug/shift are ready the moment the PE finishes group 3 ----
                def emit_mlp2_group(g):
                    for occ in range(2):
                        oc = 2 * g + occ
                        fp = fps.tile([P, 512], dt, tag="fp")
                        for jj in range(JT):
                            for hl in range(2):
                                nc.tensor.matmul(
                                    fp[0:48, :],
                                    lhsT=hT_sb[:, jj, hl, :],
                                    rhs=w2t[g][:, jj, occ * 512:(occ + 1) * 512],
                                    start=(jj == 0 and hl == 0),
                                    stop=False,
                                )
                        nc.tensor.matmul(  # + fc2_b/8 (summed to fc2_b by RS)
                            fp[0:48, :],
                            lhsT=ones_sb[:],
                            rhs=b2_sb[:, oc * 512:(oc + 1) * 512],
                            start=False,
                            stop=True,
                        )
                        fpe = fpep.tile([48, 512], dt, tag="fpe")
                        nc.vector.tensor_copy(fpe[:], fp[0:48, :])
                        nc.sync.dma_start(out=rs_in[g][:, occ, :], in_=fpe[:])
                        if occ == 1:
                            nc.gpsimd.collective_compute(
                                "ReduceScatter",
                                ALU.add,
                                replica_groups=[list(range(NCORES))],
                                ins=[rs_in[g].opt()],
                                outs=[rs_out[g].opt()],
                            )

                emit_mlp2_group(0)
                fetch_w2(2)       # reuses g0's buffer once its reads retire
                emit_mlp2_group(1)
                fetch_w2(3)
                emit_fprep(0, lambda: fps.tile([P, 512], dt, tag="fp",
                                               name="fprep0ps")[:])
                emit_mlp2_group(2)
                emit_mlp2_group(3)

            # ===================== attention =====================
            with tc.tile_pool(name="tpp", bufs=2, space="PSUM") as tpp, \
                 tc.tile_pool(name="fts2", bufs=2) as ftsp2, \
                 tc.tile_pool(name="etp", bufs=5) as etp, \
                 tc.tile_pool(name="xpp", bufs=5) as xpp, \
                 tc.tile_pool(name="accp", bufs=4) as accp:

                first_flush = [True] * NT
                ets = {}        # mb -> (et tile, xp tile)
                prev_tps = [None]
                ftsp = ftsp2  # fprep called below uses the attention pool

                def emit_mm2_chunk(pmb, ntc, dst):
                    # one n-chunk of pair (pmb, pmb+1): 2 accumulating MMs into
                    # an exp-drained PSUM sub-bank, then DVE-accumulate into O
                    for q in range(2):
                        et_q, xp_q = ets[pmb + q]
                        nc.tensor.matmul(
                            dst,
                            lhsT=xp_q[:].bitcast(dtr),
                            rhs=et_q[:, ntc, :].bitcast(dtr),
                            start=(q == 0),
                            stop=(q == 1),
                        )
                    if first_flush[ntc]:
                        nc.vector.tensor_copy(O_nt[ntc][:].bitcast(dtr), dst)
                        first_flush[ntc] = False
                    else:
                        nc.vector.tensor_add(
                            O_nt[ntc][:].bitcast(dtr), O_nt[ntc][:], dst)
                    if ntc == NT - 1:
                        del ets[pmb]
                        del ets[pmb + 1]

                def emit_slot(mb):
                    # one m-block: 2 x (2048-wide mm1 + exp); mm2 of the pair
                    # lagged by 2 m-blocks targets the previous (drained) tile
                    et = etp.tile([P, NT, 512], dt, tag="et")
                    acc = accp.tile([P, 2], dt, tag="acc")
                    lhs = faug_sb[:, mb * P:(mb + 1) * P]
                    for t in range(2):
                        tps = tpp.tile([P, 4, 512], dt, tag="tps")
                        for i in range(4):
                            nt_ = 4 * t + i
                            nc.tensor.matmul(
                                tps[:, i, :],
                                lhsT=lhs,
                                rhs=xaug_sb[:, nt_ * 512:(nt_ + 1) * 512],
                                start=True,
                                stop=True,
                            )
                        nc.scalar.activation(
                            et[:, 4 * t:4 * t + 4, :].bitcast(dtr),
                            tps[:],
                            AF.Exp,
                            scale=ls_sb[:, mb:mb + 1],
                            bias=shift_sb[:, mb:mb + 1],
                            accum_out=acc[:, t:t + 1],
                        )
                        if mb >= 2:
                            pmb = (mb // 2 - 1) * 2
                            base = (mb % 2) * 4 + 2 * t
                            for i2 in range(2):
                                emit_mm2_chunk(
                                    pmb, base + i2,
                                    prev_tps[0][0:C, 2 * t + i2, :])
                        prev_tps[0] = tps
                    cs = accp.tile([P, 1], dt, tag="cs")
                    nc.vector.reduce_sum(cs[:], acc[:], AX.X)
                    rc = accp.tile([P, 1], dt, tag="rc")
                    nc.vector.reciprocal(rc[:], cs[:])
                    xp = xpp.tile([P, C], dt, tag="xp")
                    nc.vector.tensor_scalar_mul(
                        xp[:].bitcast(dtr), xtb_sb[:, mb, :], rc[:])
                    ets[mb] = (et, xp)

                def tpp_ptile():
                    t = tpp.tile([P, 4, 512], dt, tag="tps", name="tpsx")
                    return t[:, 0, :]

                # fprep(0) already ran during MLP2; prep group g+1 one group
                # early so its faug DMAs never stall the PE at a boundary
                for g in range(NGRP):
                    if g + 1 < NGRP:
                        emit_fprep(g + 1, tpp_ptile)
                    for mb in range(g * MB_PER_G, (g + 1) * MB_PER_G):
                        emit_slot(mb)
                # drain: last pair's mm2 chunks
                for d in range(2):
                    tps = tpp.tile([P, 4, 512], dt, tag="tps")
                    for i in range(4):
                        emit_mm2_chunk(MBS - 2, 4 * d + i, tps[0:C, i, :])

                # stack the 8 [6,512] chunks into [48,512] (partition-offset
                # SBUF->SBUF DMAs)
                for nt_ in range(NT):
                    nc.sync.dma_start(
                        out=O48_sb[C * nt_:C * nt_ + C, :].bitcast(dtr),
                        in_=O_nt[nt_][:].bitcast(dtr))

            # ===================== LayerNorm over c + output =====================
            with tc.tile_pool(name="lnps", bufs=2, space="PSUM") as lnps, \
                 tc.tile_pool(name="lnrp", bufs=2, space="PSUM") as lnrp, \
                 tc.tile_pool(name="lnsb", bufs=1) as lnsb:
                eps_sb = lnsb.tile([NT, 1], dt, tag="eps")
                nc.vector.memset(eps_sb[:], EPS)
                O2_sb = lnsb.tile([48, 512], dt, tag="O2")
                nc.vector.tensor_mul(O2_sb[:].bitcast(dtr), O48_sb[:], O48_sb[:])
                s_ps = lnps.tile([NT, 512], dt, tag="sps")
                nc.tensor.matmul(
                    s_ps[:], lhsT=blk_sb[:], rhs=O48_sb[:].bitcast(dtr),
                    start=True, stop=True,
                )
                s2_ps = lnps.tile([NT, 512], dt, tag="s2ps")
                nc.tensor.matmul(
                    s2_ps[:], lhsT=blk_sb[:], rhs=O2_sb[:].bitcast(dtr),
                    start=True, stop=True,
                )
                mean_sb = lnsb.tile([NT, 512], dt, tag="mean")
                nc.vector.tensor_scalar_mul(
                    mean_sb[:].bitcast(dtr), s_ps[:], 1.0 / C)
                ms_sb = lnsb.tile([NT, 512], dt, tag="ms")
                nc.vector.tensor_mul(ms_sb[:], mean_sb[:], mean_sb[:])
                var_sb = lnsb.tile([NT, 512], dt, tag="var")
                nc.vector.tensor_scalar_mul(var_sb[:], s2_ps[:], 1.0 / C)
                nc.vector.tensor_sub(var_sb[:], var_sb[:], ms_sb[:])
                # 1/sqrt(var+eps) via ln/exp (same resident table set)
                vln_sb = lnsb.tile([NT, 512], dt, tag="vln")
                nc.scalar.activation(vln_sb[:], var_sb[:], AF.Ln, bias=eps_sb[:])
                rstd_sb = lnsb.tile([NT, 512], dt, tag="rstd")
                nc.scalar.activation(rstd_sb[:].bitcast(dtr), vln_sb[:], AF.Exp,
                                     scale=-0.5, bias=zero128_sb[0:NT, :])
                mrep = lnrp.tile([48, 512], dt, tag="mrep")
                nc.tensor.matmul(
                    mrep[:], lhsT=blkT_sb[:], rhs=mean_sb[:].bitcast(dtr),
                    start=True, stop=True,
                )
                rrep = lnrp.tile([48, 512], dt, tag="rrep")
                nc.tensor.matmul(
                    rrep[:], lhsT=blkT_sb[:], rhs=rstd_sb[:].bitcast(dtr),
                    start=True, stop=True,
                )
                on_sb = lnsb.tile([48, 512], dt, tag="on")
                nc.vector.tensor_sub(on_sb[:], O48_sb[:], mrep[:])
                nc.vector.tensor_mul(on_sb[:], on_sb[:], rrep[:])
                nc.vector.tensor_scalar(
                    out=on_sb[:], in0=on_sb[:],
                    scalar1=wb48_sb[:, 0:1], scalar2=wb48_sb[:, 1:2],
                    op0=ALU.mult, op1=ALU.add,
                )
                for nt_ in range(NT):
                    nc.sync.dma_start(
                        out=out_d[:, nt_ * 512:(nt_ + 1) * 512],
                        in_=on_sb[C * nt_:C * nt_ + C, :],
                    )

    nc.compile()
    return nc


def _host_prep(inputs):
    import ml_dtypes
    bf16 = ml_dtypes.bfloat16

    x = np.asarray(inputs["x"], np.float32)
    feature = np.asarray(inputs["feature"], np.float32)
    fc1_w = np.asarray(inputs["fc1_w"], np.float32)
    fc1_b = np.asarray(inputs["fc1_b"], np.float32)
    fc2_w = np.asarray(inputs["fc2_w"], np.float32)
    fc2_b = np.asarray(inputs["fc2_b"], np.float32)
    logit_scale = np.asarray(inputs["logit_scale"], np.float32)
    norm_w = np.asarray(inputs["norm_w"], np.float32)
    norm_b = np.asarray(inputs["norm_b"], np.float32)

    def split_hl(a, dtyp):
        hi = a.astype(dtyp)
        lo = (a - hi.astype(np.float32)).astype(dtyp)
        return hi, lo

    w1T = np.ascontiguousarray(fc1_w.T)                      # [f, h]
    w2T = np.ascontiguousarray(fc2_w.T)                      # [h, o]
    featT = np.ascontiguousarray(feature.reshape(B * C, FF).T)   # [f, bc]
    fth, ftl = split_hl(featT, np.float16)
    # [128, 72, 2, 48]
    featT_b = np.ascontiguousarray(
        np.stack([fth, ftl], axis=1).reshape(KT1, P, 2, B * C)
        .transpose(1, 0, 2, 3))
    ls = np.exp(np.minimum(logit_scale.reshape(HW), np.log(np.float32(100.0))))
    ls_b = np.ascontiguousarray(ls.reshape(MBS, P).T).astype(np.float32)
    ones1 = np.ones((1, B * C), np.float32)
    id48 = np.eye(48, dtype=np.float32)
    blk = np.zeros((48, NT), np.float32)
    blk[np.arange(48), np.arange(48) // C] = 1.0
    blkT = np.ascontiguousarray(blk.T)
    wb48 = np.ascontiguousarray(
        np.stack([np.tile(norm_w, NT), np.tile(norm_b, NT)], axis=1))
    b2 = (fc2_b / NCORES).reshape(1, HW).astype(np.float32)

    in_maps = []
    for k in range(NCORES):
        w1k = w1T[:, k * HS:(k + 1) * HS].astype(np.float16)   # [9216, 1152]
        # [24, 128, 3, 1152]
        w1s = np.ascontiguousarray(
            w1k.reshape(NC1, CH1, P, HS).transpose(0, 2, 1, 3))
        b1k = np.ascontiguousarray(fc1_b[k * HS:(k + 1) * HS]).reshape(1, HS)
        w2k = w2T[k * HS:(k + 1) * HS, :].astype(np.float16)   # [1152, 4096]
        # [4, 128, 9, 1024]
        w2s = np.ascontiguousarray(
            w2k.reshape(JT, P, NGRP, 1024).transpose(2, 1, 0, 3))
        xh, xl = split_hl(x[k], bf16)                          # [6, 4096]
        xaug = np.concatenate([xh, xl, xh, xl], axis=0)        # [24, 4096]
        xtbk = np.ascontiguousarray(x[k].T.reshape(MBS, P, C).transpose(1, 0, 2))
        xmax_k = np.linalg.norm(x[k], axis=0).max()
        # 2^63.5 rescale of the bitcast-sqrt seed folded in
        negx = np.full((P, 1), -xmax_k * 2.0**63.5, np.float32)
        srt = np.ascontiguousarray(
            np.broadcast_to(np.array([[1, 0x1FBD1DF5]], np.int32), (P, 2)))
        in_maps.append({
            "featT": featT_b, "w1t": w1s, "b1": b1k, "w2t": w2s, "b2": b2,
            "xaug": xaug, "xtb": xtbk, "lsb": ls_b, "ones1": ones1,
            "id48": id48, "blk": blk, "blkT": blkT, "wb48": wb48,
            "negx": negx, "srtc": srt,
        })
    return in_maps


def _install_ntff_shim():
    # The agent image's `antenv` lacks `axon_hooks`, which bass_utils needs
    # for trace=True under axon. Fabricate the registry module and install
    # the ctypes-based NTFF hook against libaxon_pjrt.so.
    import sys
    import types
    import ctypes
    import contextlib

    try:
        import antenv.axon_hooks  # noqa: F401
        return
    except ImportError:
        pass
    if "antenv.axon_hooks" in sys.modules:
        return
    mod = types.ModuleType("antenv.axon_hooks")
    _h = [None]
    mod.set_axon_ntff_profile_hook = lambda h: _h.__setitem__(0, h)
    mod.get_axon_ntff_profile_hook = lambda: _h[0]
    sys.modules["antenv.axon_hooks"] = mod

    so_path = "/opt/axon/libaxon_pjrt.so"
    if not os.path.exists(so_path):
        return
    lib = ctypes.CDLL(so_path)
    if not hasattr(lib, "axon_start_nrt_profile"):
        return
    lib.axon_start_nrt_profile.argtypes = [
        ctypes.POINTER(ctypes.c_int64), ctypes.c_size_t]
    lib.axon_start_nrt_profile.restype = ctypes.c_int64
    lib.axon_stop_nrt_profile.argtypes = [ctypes.c_char_p]
    lib.axon_stop_nrt_profile.restype = ctypes.c_int64

    @contextlib.contextmanager
    def _hook(output_dir, device_ids):
        import jax
        jax.devices()
        if device_ids:
            ids = (ctypes.c_int64 * len(device_ids))(*device_ids)
            rc = lib.axon_start_nrt_profile(ids, len(device_ids))
        else:
            rc = lib.axon_start_nrt_profile(None, 0)
        if rc != 0:
            raise RuntimeError(f"axon_start_nrt_profile rc={rc}")
        try:
            yield
        finally:
            n = lib.axon_stop_nrt_profile(str(output_dir).encode())
            print(f"ntff profile: {n} file(s) written to {output_dir}")

    mod.set_axon_ntff_profile_hook(_hook)


def kernel(**inputs):
    from concourse.bass_utils import run_bass_kernel_spmd

    if bool(int(os.environ.get("BASS_KT_TRACE", "0"))):
        _install_ntff_shim()

    if "nc" not in _cache:
        _cache["nc"] = _build_program()
    nc = _cache["nc"]

    in_maps = _host_prep(inputs)
    trace = bool(int(os.environ.get("BASS_KT_TRACE", "0")))
    res = run_bass_kernel_spmd(nc, in_maps, list(range(NCORES)), trace=trace)
    kernel.last_results = res
    out = np.stack([np.asarray(res.results[k]["out"]) for k in range(NCORES)])
    return out.astype(np.float32)


# revision 22
# speedup vs baseline: 1.2810x; 1.2810x over previous
# kernel.py — Trainium2 Bass kernel for nn_ChannelAttentionBlock (v3)
#
# Computation (per reference):
#   h = relu(feature @ fc1_w.T + fc1_b)            [B,C,FF]
#   f = h @ fc2_w.T + fc2_b                        [B,C,HW]
#   T[b,n,m] = sum_c x[b,c,n] * f[b,c,m] * ls[m]   (ls = exp(min(logit_scale, log 100)))
#   P = softmax_n(T);  out[b,n,c] = sum_m P[n,m] x[b,c,m];  LayerNorm over c; -> [B,C,HW]
#
# Sharding (8 cores):
#   MLP tensor-parallel: fc1 split on hidden (each core 1152 of 9216 hidden),
#   fc2 split on input; ReduceScatter over batch so core k receives f[b=k].
#   Attention data-parallel: core k handles batch k entirely.
#
# Precision (PE streams 1 col/cyc for bf16/fp16, fp32r is 1 cyc when N>=256):
#   MLP1/MLP2: fp16 weights single-plane + fp16 hi/lo activations, 2 pairings
#     (fh.w + fl.w) — measured 7e-3 rel err vs the 2e-2 budget. Halves w1 DMA
#     (21.2MB/core) and cuts MLP PE work by a third vs bf16 3-way.
#   mm1 (T = f^T x, K=24): bf16 hi/lo K-stacked — full precision, free in K.
#   mm2 (out += xp^T E): fp32r on fp32 et (1cyc/row at N=512).
#   Softmax shift: per-m Cauchy-Schwarz bound -ls*xmax*||f[:,m]||+40.
#
# Schedule (v3): phase-separated for PE clock-gate hygiene. The PE drops to
# 1.2 GHz after a >3.4us idle and only re-promotes after ~3us of gapless
# execution, so work is organized in long wait-free streaks:
#   [w1 DMA || MLP1] -> [MLP2 all 4 groups back-to-back; RS(g) issued as each
#   group finishes, RS wait hidden under later groups] -> [attention: 32
#   m-block slots; 2048-wide exp on 2x[128,4,512] PSUM rotation (all 8 banks);
#   mm2 of the lagged pair writes into exp-drained sub-banks of the previous
#   tile; per-slot PE burst ~3.5us vs ACT 4.2us] -> LayerNorm.

import os
import numpy as np

B, C, HW, FF, P = 8, 6, 4096, 9216, 128
NCORES = 8
HS = FF // NCORES        # 1152
KT1 = FF // P            # 72 fc1 K tiles
CH1 = 3                  # fc1 K tiles per DMA chunk
NC1 = KT1 // CH1         # 24 fc1 chunks
JT = HS // P             # 9  fc2 K tiles
NT = HW // 512           # 8  512-wide n/o chunks
MBS = HW // P            # 32 m blocks
NGRP = 4                 # reduce-scatter groups (2 o-chunks each)
MB_PER_G = MBS // NGRP   # 8
EPS = 1e-5
SHIFT_HEADROOM = 40.0

_cache = {}


def _build_program():
    import concourse.bacc as bacc
    import concourse.bass as bass
    import concourse.tile as tile
    import concourse.mybir as mybir

    dt = mybir.dt.float32
    dtr = mybir.dt.float32r
    dtb = mybir.dt.bfloat16
    dth = mybir.dt.float16
    AF = mybir.ActivationFunctionType
    ALU = mybir.AluOpType
    AX = mybir.AxisListType

    nc = bacc.Bacc(
        "TRN2",
        target_bir_lowering=False,
        debug=False,
        enable_asserts=False,
        num_devices=NCORES,
    )

    # ---- external I/O ----
    featT_d = nc.dram_tensor("featT", [P, KT1, 2, 48], dth, kind="ExternalInput").ap()
    w1_d = nc.dram_tensor("w1t", [NC1, P, CH1, HS], dth, kind="ExternalInput").ap()
    b1_d = nc.dram_tensor("b1", [1, HS], dt, kind="ExternalInput").ap()
    w2_d = nc.dram_tensor("w2t", [NGRP, P, JT, 1024], dth, kind="ExternalInput").ap()
    b2_d = nc.dram_tensor("b2", [1, HW], dt, kind="ExternalInput").ap()  # fc2_b/8
    xaug_d = nc.dram_tensor("xaug", [24, HW], dtb, kind="ExternalInput").ap()
    xtb_d = nc.dram_tensor("xtb", [P, MBS, C], dt, kind="ExternalInput").ap()
    ls_d = nc.dram_tensor("lsb", [P, MBS], dt, kind="ExternalInput").ap()
    srt_d = nc.dram_tensor("srtc", [P, 2], mybir.dt.int32,
                           kind="ExternalInput").ap()
    ones_d = nc.dram_tensor("ones1", [1, 48], dt, kind="ExternalInput").ap()
    id48_d = nc.dram_tensor("id48", [48, 48], dt, kind="ExternalInput").ap()
    blk_d = nc.dram_tensor("blk", [48, NT], dt, kind="ExternalInput").ap()
    blkT_d = nc.dram_tensor("blkT", [NT, 48], dt, kind="ExternalInput").ap()
    wb48_d = nc.dram_tensor("wb48", [48, 2], dt, kind="ExternalInput").ap()
    negx_d = nc.dram_tensor("negx", [P, 1], dt, kind="ExternalInput").ap()
    out_d = nc.dram_tensor("out", [C, HW], dt, kind="ExternalOutput").ap()

    with tile.TileContext(nc) as tc:
        # float32r APs carry full-fp32 bit patterns; the PE rounds at load.
        with nc.allow_low_precision(reason="fp32r/fp16/bf16 kernel dataflow"), \
             tc.tile_pool(name="const", bufs=1) as const, \
             tc.tile_pool(name="dram", bufs=1, space="DRAM") as dram:

            # ---- constants / small inputs ----
            xaug_sb = const.tile([24, HW], dtb, tag="xaug")
            nc.gpsimd.dma_start(out=xaug_sb[:], in_=xaug_d)
            xtb_sb = const.tile([P, MBS, C], dt, tag="xtb")
            nc.gpsimd.dma_start(out=xtb_sb[:], in_=xtb_d)
            ls_sb = const.tile([P, MBS], dt, tag="ls")
            nc.gpsimd.dma_start(out=ls_sb[:], in_=ls_d)
            ones_sb = const.tile([1, 48], dtr, tag="ones1")
            nc.gpsimd.dma_start(out=ones_sb[:], in_=ones_d.bitcast(dtr))
            id48_sb = const.tile([48, 48], dt, tag="id48")
            nc.gpsimd.dma_start(out=id48_sb[:], in_=id48_d)
            negx_sb = const.tile([P, 1], dt, tag="negx")
            nc.gpsimd.dma_start(out=negx_sb[:], in_=negx_d)
            srt_sb = const.tile([P, 2], mybir.dt.int32, tag="srtc")
            nc.gpsimd.dma_start(out=srt_sb[:], in_=srt_d)
            blk_sb = const.tile([48, NT], dtr, tag="blk")
            nc.gpsimd.dma_start(out=blk_sb[:], in_=blk_d.bitcast(dtr))
            blkT_sb = const.tile([NT, 48], dtr, tag="blkT")
            nc.gpsimd.dma_start(out=blkT_sb[:], in_=blkT_d.bitcast(dtr))
            wb48_sb = const.tile([48, 2], dt, tag="wb48")
            nc.gpsimd.dma_start(out=wb48_sb[:], in_=wb48_d)

            # h^T in fp16 hi/lo planes (filled after MLP1)
            hT_sb = const.tile([P, JT, 2, 48], dth, tag="hT")
            faug_sb = const.tile([24, HW], dtb, tag="faug")
            # per-m softmax shift (exp bias): -ls*xmax*||f[:,m]|| + 40
            shift_sb = const.tile([P, MBS], dt, tag="shift")
            zero128_sb = const.tile([P, 1], dt, tag="zero128")
            nc.vector.memset(zero128_sb[:], 0.0)
            ones6_sb = const.tile([C, 1], dtb, tag="ones6")
            nc.vector.memset(ones6_sb[:], 1.0)
            # attention output accumulators, one [6,512] tile per n-chunk
            O_nt = [const.tile([C, 512], dt, tag=f"O{nt}", name=f"O{nt}")
                    for nt in range(NT)]
            wz_sb = const.tile([1, 640], dt, tag="wz")
            nc.vector.memset(wz_sb[:], 0.0)
            O48_sb = const.tile([48, 512], dt, tag="O48")

            # DRAM bounce buffers for the reduce-scatter groups
            rs_in = [dram.tile([48, 2, 512], dt, tag=f"rsin{g}", name=f"rsin{g}")
                     for g in range(NGRP)]
            rs_out = [dram.tile([C, 2, 512], dt, tag=f"rsout{g}", name=f"rsout{g}")
                      for g in range(NGRP)]
            wrm_in = dram.tile([1, 8], dt, tag="wrmin", name="wrmin")
            wrm_out = dram.tile([1, 8], dt, tag="wrmout", name="wrmout")

            # ============ warmup: PE clock promote, exp tables, CC cores ============
            with tc.tile_pool(name="wrm", bufs=1, space="PSUM") as wrmp:
                wps = wrmp.tile([P, 512], dt, tag="wps")
                for _ in range(12):
                    nc.tensor.matmul(
                        wps[:],
                        lhsT=wz_sb[:, 0:P].bitcast(dtr),
                        rhs=wz_sb[:, P:P + 512].bitcast(dtr),
                        start=True,
                        stop=True,
                    )
                # load the exp table set now (Exp+Relu share it; the kernel
                # body uses no other table until the final LayerNorm)
                wex = wrmp.tile([1, 8], dt, tag="wex")
                nc.scalar.activation(wex[:], wz_sb[:, 0:8], AF.Exp,
                                     bias=zero128_sb[0:1, :])
            # init + warm the collective path (RDH state, cross-core skew)
            nc.gpsimd.dma_start(out=wrm_in[:], in_=wz_sb[:, 0:8])
            nc.gpsimd.collective_compute(
                "AllReduce",
                ALU.add,
                replica_groups=[list(range(NCORES))],
                ins=[wrm_in.opt()],
                outs=[wrm_out.opt()],
            )

            # ====== MLP1 (w2 DMAs interleaved) / MLP2 + RS / fprep(0) ======
            with tc.tile_pool(name="w2p", bufs=NGRP) as w2p, \
                 tc.tile_pool(name="m2c", bufs=1) as m2c, \
                 tc.tile_pool(name="fts", bufs=2) as ftsp, \
                 tc.tile_pool(name="fps", bufs=2, space="PSUM") as fps, \
                 tc.tile_pool(name="fpe", bufs=2) as fpep:

                def emit_fprep(g, get_ptile):
                    # runs once RS(g) lands: distribute f, build faug + shift.
                    # get_ptile() -> [P, 512] PSUM AP for the norm matmuls.
                    gsl = slice(g * 1024, (g + 1) * 1024)
                    fgrp = ftsp.tile([C, 1024], dt, tag="fgrp")
                    nc.sync.dma_start(out=fgrp[:], in_=rs_out[g][:])
                    # bf16 hi/lo split of f for mm1, rows [fh,fh,fl,fl]
                    fh = ftsp.tile([C, 1024], dtb, tag="fh")
                    nc.vector.tensor_copy(fh[:], fgrp[:])
                    fl = ftsp.tile([C, 1024], dtb, tag="fl")
                    nc.vector.tensor_sub(fl[:], fgrp[:], fh[:])
                    nc.sync.dma_start(out=faug_sb[0:C, gsl], in_=fh[:])
                    nc.sync.dma_start(out=faug_sb[C:2 * C, gsl], in_=fh[:])
                    nc.sync.dma_start(out=faug_sb[2 * C:3 * C, gsl], in_=fl[:])
                    nc.sync.dma_start(out=faug_sb[3 * C:24, gsl], in_=fl[:])
                    # shift[m] = -ls[m]*xmax*||f[:,m]||_2 + 40; column sq-norms
                    # via tiny matmuls into a borrowed PSUM tile, then sqrt on
                    # DVE via the bitcast seed (+-3.5%, harmless in a bound
                    # with ~47 e-folds of margin). No ACT => no table reloads.
                    fsq = ftsp.tile([C, 1024], dtb, tag="fsq")
                    nc.vector.tensor_mul(fsq[:], fgrp[:], fgrp[:])
                    pt = get_ptile()
                    for chn in range(NT):
                        nc.tensor.matmul(
                            pt[:, chn:chn + 1],
                            lhsT=fsq[:, chn * P:(chn + 1) * P],
                            rhs=ones6_sb[:],
                            start=True,
                            stop=True,
                        )
                    # y = bitcast(i >> 1) = sqrt(S) * 2^-63.5 * (1..1.061);
                    # the 2^63.5 rescale is folded into negx on the host
                    fns = ftsp.tile([P, NT], mybir.dt.int32, tag="fns")
                    nc.vector.tensor_scalar(
                        out=fns[:], in0=pt[:, 0:NT].bitcast(mybir.dt.int32),
                        scalar1=srt_sb[:, 0:1], scalar2=0.0,
                        op0=ALU.arith_shift_right, op1=ALU.bypass,
                    )
                    fnl = ftsp.tile([P, NT], dt, tag="fnl")
                    nc.vector.tensor_mul(
                        fnl[:], fns[:].bitcast(dt),
                        ls_sb[:, NT * g:NT * (g + 1)])
                    nc.vector.tensor_scalar(
                        out=shift_sb[:, NT * g:NT * (g + 1)], in0=fnl[:],
                        scalar1=negx_sb[:], scalar2=SHIFT_HEADROOM,
                        op0=ALU.mult, op1=ALU.add,
                    )

                b2_sb = m2c.tile([1, HW], dtr, tag="b2")
                nc.gpsimd.dma_start(out=b2_sb[:], in_=b2_d.bitcast(dtr))
                w2t = {}

                def fetch_w2(g):
                    w2t[g] = w2p.tile([P, JT, 1024], dth, tag="w2t",
                                      name=f"w2t{g}")
                    nc.sync.dma_start(out=w2t[g][:], in_=w2_d[g])



                # ---- MLP1: h = relu(feat @ w1 + b1), fp16 2-way ----
                with tc.tile_pool(name="w1p", bufs=4) as w1p, \
                     tc.tile_pool(name="m1c", bufs=1) as m1c, \
                     tc.tile_pool(name="ps1", bufs=1, space="PSUM") as ps1, \
                     tc.tile_pool(name="pst", bufs=2, space="PSUM") as pst:
                    featT_sb = m1c.tile([P, KT1, 2, 48], dth, tag="featT")
                    nc.gpsimd.dma_start(out=featT_sb[:], in_=featT_d)
                    b1_sb = m1c.tile([1, HS], dtr, tag="b1")
                    nc.gpsimd.dma_start(out=b1_sb[:], in_=b1_d.bitcast(dtr))
                    h_sb = m1c.tile([48, HS], dt, tag="h")
                    hp = ps1.tile([48, 3, 512], dt, tag="hp")
                    for ch in range(NC1):
                        w1t = w1p.tile([P, CH1, HS], dth, tag="w1t")
                        nc.sync.dma_start(out=w1t[:], in_=w1_d[ch])
                        if ch == NC1 - 1:
                            # w2 g0 streams beside the last w1 chunk so the
                            # MLP1->MLP2 PE gap stays under the demote window
                            fetch_w2(0)
                        for s in range(CH1):
                            kk = CH1 * ch + s
                            for hl in range(2):
                                lhs = featT_sb[:, kk, hl, :]
                                for j in range(3):
                                    nc.tensor.matmul(
                                        hp[:, j, 0:384],
                                        lhsT=lhs,
                                        rhs=w1t[:, s, j * 384:(j + 1) * 384],
                                        start=(kk == 0 and hl == 0),
                                        stop=False,
                                    )
                    for j in range(3):  # bias via K=1 ones row (fp32r)
                        nc.tensor.matmul(
                            hp[:, j, 0:384],
                            lhsT=ones_sb[:],
                            rhs=b1_sb[:, j * 384:(j + 1) * 384],
                            start=False,
                            stop=True,
                        )
                    for j in range(3):  # relu eviction on ACT
                        nc.scalar.activation(
                            h_sb[:, j * 384:(j + 1) * 384],
                            hp[:, j, 0:384],
                            AF.Relu,
                        )
                    # transpose h -> hT (9 PE transposes), split fp16 hi/lo
                    for t in range(JT):
                        tp = pst.tile([P, 48], dt, tag="tp")
                        nc.tensor.transpose(
                            tp[:], h_sb[:, t * P:(t + 1) * P], id48_sb[:]
                        )
                        nc.vector.tensor_copy(hT_sb[:, t, 0, :], tp[:])
                        nc.vector.tensor_sub(hT_sb[:, t, 1, :], tp[:],
                                             hT_sb[:, t, 0, :])

                # ---- MLP2 groups + RS; fprep(0) tucked after group 1 so its
                # faug/shift are ready the moment the PE finishes group 3 ----
                def emit_mlp2_group(g):
                    for occ in range(2):
                        oc = 2 * g + occ
                        fp = fps.tile([P, 512], dt, tag="fp")
                        for jj in range(JT):
                            for hl in range(2):
                                nc.tensor.matmul(
                                    fp[0:48, :],
                                    lhsT=hT_sb[:, jj, hl, :],
                                    rhs=w2t[g][:, jj, occ * 512:(occ + 1) * 512],
                                    start=(jj == 0 and hl == 0),
                                    stop=False,
                                )
                        nc.tensor.matmul(  # + fc2_b/8 (summed to fc2_b by RS)
                            fp[0:48, :],
                            lhsT=ones_sb[:],
                            rhs=b2_sb[:, oc * 512:(oc + 1) * 512],
                            start=False,
                            stop=True,
                        )
                        fpe = fpep.tile([48, 512], dt, tag="fpe")
                        nc.vector.tensor_copy(fpe[:], fp[0:48, :])
                        nc.sync.dma_start(out=rs_in[g][:, occ, :], in_=fpe[:])
                        if occ == 1:
                            nc.gpsimd.collective_compute(
                                "ReduceScatter",
                                ALU.add,
                                replica_groups=[list(range(NCORES))],
                                ins=[rs_in[g].opt()],
                                outs=[rs_out[g].opt()],
                            )

                fetch_w2(1)
                emit_mlp2_group(0)
                fetch_w2(2)       # reuses g0's buffer once its reads retire
                emit_mlp2_group(1)
                fetch_w2(3)
                emit_fprep(0, lambda: fps.tile([P, 512], dt, tag="fp",
                                               name="fprep0ps")[:])
                emit_mlp2_group(2)
                emit_mlp2_group(3)

            # ===================== attention =====================
            with tc.tile_pool(name="tpp", bufs=2, space="PSUM") as tpp, \
                 tc.tile_pool(name="fts2", bufs=2) as ftsp2, \
                 tc.tile_pool(name="etp", bufs=5) as etp, \
                 tc.tile_pool(name="xpp", bufs=5) as xpp, \
                 tc.tile_pool(name="accp", bufs=4) as accp:

                first_flush = [True] * NT
                ets = {}        # mb -> (et tile, xp tile)
                prev_tps = [None]
                ftsp = ftsp2  # fprep called below uses the attention pool

                def emit_mm2_chunk(pmb, ntc, dst):
                    # one n-chunk of pair (pmb, pmb+1): 2 accumulating MMs into
                    # an exp-drained PSUM sub-bank, then DVE-accumulate into O
                    for q in range(2):
                        et_q, xp_q = ets[pmb + q]
                        nc.tensor.matmul(
                            dst,
                            lhsT=xp_q[:].bitcast(dtr),
                            rhs=et_q[:, ntc, :].bitcast(dtr),
                            start=(q == 0),
                            stop=(q == 1),
                        )
                    if first_flush[ntc]:
                        nc.vector.tensor_copy(O_nt[ntc][:].bitcast(dtr), dst)
                        first_flush[ntc] = False
                    else:
                        nc.vector.tensor_add(
                            O_nt[ntc][:].bitcast(dtr), O_nt[ntc][:], dst)
                    if ntc == NT - 1:
                        del ets[pmb]
                        del ets[pmb + 1]

                def emit_slot(mb):
                    # one m-block: 2 x (2048-wide mm1 + exp); mm2 of the pair
                    # lagged by 2 m-blocks targets the previous (drained) tile
                    et = etp.tile([P, NT, 512], dt, tag="et")
                    acc = accp.tile([P, 2], dt, tag="acc")
                    lhs = faug_sb[:, mb * P:(mb + 1) * P]
                    for t in range(2):
                        tps = tpp.tile([P, 4, 512], dt, tag="tps")
                        for i in range(4):
                            nt_ = 4 * t + i
                            nc.tensor.matmul(
                                tps[:, i, :],
                                lhsT=lhs,
                                rhs=xaug_sb[:, nt_ * 512:(nt_ + 1) * 512],
                                start=True,
                                stop=True,
                            )
                        nc.scalar.activation(
                            et[:, 4 * t:4 * t + 4, :].bitcast(dtr),
                            tps[:],
                            AF.Exp,
                            scale=ls_sb[:, mb:mb + 1],
                            bias=shift_sb[:, mb:mb + 1],
                            accum_out=acc[:, t:t + 1],
                        )
                        if mb >= 2:
                            pmb = (mb // 2 - 1) * 2
                            base = (mb % 2) * 4 + 2 * t
                            for i2 in range(2):
                                emit_mm2_chunk(
                                    pmb, base + i2,
                                    prev_tps[0][0:C, 2 * t + i2, :])
                        prev_tps[0] = tps
                    cs = accp.tile([P, 1], dt, tag="cs")
                    nc.vector.reduce_sum(cs[:], acc[:], AX.X)
                    rc = accp.tile([P, 1], dt, tag="rc")
                    nc.vector.reciprocal(rc[:], cs[:])
                    xp = xpp.tile([P, C], dt, tag="xp")
                    nc.vector.tensor_scalar_mul(
                        xp[:].bitcast(dtr), xtb_sb[:, mb, :], rc[:])
                    ets[mb] = (et, xp)

                def tpp_ptile():
                    t = tpp.tile([P, 4, 512], dt, tag="tps", name="tpsx")
                    return t[:, 0, :]

                # fprep(0) already ran during MLP2; prep group g+1 one group
                # early so its faug DMAs never stall the PE at a boundary
                for g in range(NGRP):
                    if g + 1 < NGRP:
                        emit_fprep(g + 1, tpp_ptile)
                    for mb in range(g * MB_PER_G, (g + 1) * MB_PER_G):
                        emit_slot(mb)
                # drain: last pair's mm2 chunks
                for d in range(2):
                    tps = tpp.tile([P, 4, 512], dt, tag="tps")
                    for i in range(4):
                        emit_mm2_chunk(MBS - 2, 4 * d + i, tps[0:C, i, :])

                # stack the 8 [6,512] chunks into [48,512] (partition-offset
                # SBUF->SBUF DMAs)
                for nt_ in range(NT):
                    nc.sync.dma_start(
                        out=O48_sb[C * nt_:C * nt_ + C, :].bitcast(dtr),
                        in_=O_nt[nt_][:].bitcast(dtr))

            # ===================== LayerNorm over c + output =====================
            with tc.tile_pool(name="lnps", bufs=2, space="PSUM") as lnps, \
                 tc.tile_pool(name="lnrp", bufs=2, space="PSUM") as lnrp, \
                 tc.tile_pool(name="lnsb", bufs=1) as lnsb:
                eps_sb = lnsb.tile([NT, 1], dt, tag="eps")
                nc.vector.memset(eps_sb[:], EPS)
                O2_sb = lnsb.tile([48, 512], dt, tag="O2")
                nc.vector.tensor_mul(O2_sb[:].bitcast(dtr), O48_sb[:], O48_sb[:])
                s_ps = lnps.tile([NT, 512], dt, tag="sps")
                nc.tensor.matmul(
                    s_ps[:], lhsT=blk_sb[:], rhs=O48_sb[:].bitcast(dtr),
                    start=True, stop=True,
                )
                s2_ps = lnps.tile([NT, 512], dt, tag="s2ps")
                nc.tensor.matmul(
                    s2_ps[:], lhsT=blk_sb[:], rhs=O2_sb[:].bitcast(dtr),
                    start=True, stop=True,
                )
                mean_sb = lnsb.tile([NT, 512], dt, tag="mean")
                nc.vector.tensor_scalar_mul(
                    mean_sb[:].bitcast(dtr), s_ps[:], 1.0 / C)
                ms_sb = lnsb.tile([NT, 512], dt, tag="ms")
                nc.vector.tensor_mul(ms_sb[:], mean_sb[:], mean_sb[:])
                var_sb = lnsb.tile([NT, 512], dt, tag="var")
                nc.vector.tensor_scalar_mul(var_sb[:], s2_ps[:], 1.0 / C)
                nc.vector.tensor_sub(var_sb[:], var_sb[:], ms_sb[:])
                # 1/sqrt(var+eps) via ln/exp (same resident table set)
                vln_sb = lnsb.tile([NT, 512], dt, tag="vln")
                nc.scalar.activation(vln_sb[:], var_sb[:], AF.Ln, bias=eps_sb[:])
                rstd_sb = lnsb.tile([NT, 512], dt, tag="rstd")
                nc.scalar.activation(rstd_sb[:].bitcast(dtr), vln_sb[:], AF.Exp,
                                     scale=-0.5, bias=zero128_sb[0:NT, :])
                mrep = lnrp.tile([48, 512], dt, tag="mrep")
                nc.tensor.matmul(
                    mrep[:], lhsT=blkT_sb[:], rhs=mean_sb[:].bitcast(dtr),
                    start=True, stop=True,
                )
                rrep = lnrp.tile([48, 512], dt, tag="rrep")
                nc.tensor.matmul(
                    rrep[:], lhsT=blkT_sb[:], rhs=rstd_sb[:].bitcast(dtr),
                    start=True, stop=True,
                )
                on_sb = lnsb.tile([48, 512], dt, tag="on")
                nc.vector.tensor_sub(on_sb[:], O48_sb[:], mrep[:])
                nc.vector.tensor_mul(on_sb[:], on_sb[:], rrep[:])
                nc.vector.tensor_scalar(
                    out=on_sb[:], in0=on_sb[:],
                    scalar1=wb48_sb[:, 0:1], scalar2=wb48_sb[:, 1:2],
                    op0=ALU.mult, op1=ALU.add,
                )
                for nt_ in range(NT):
                    nc.sync.dma_start(
                        out=out_d[:, nt_ * 512:(nt_ + 1) * 512],
                        in_=on_sb[C * nt_:C * nt_ + C, :],
                    )

    nc.compile()
    return nc


def _host_prep(inputs):
    import ml_dtypes
    bf16 = ml_dtypes.bfloat16

    x = np.asarray(inputs["x"], np.float32)
    feature = np.asarray(inputs["feature"], np.float32)
    fc1_w = np.asarray(inputs["fc1_w"], np.float32)
    fc1_b = np.asarray(inputs["fc1_b"], np.float32)
    fc2_w = np.asarray(inputs["fc2_w"], np.float32)
    fc2_b = np.asarray(inputs["fc2_b"], np.float32)
    logit_scale = np.asarray(inputs["logit_scale"], np.float32)
    norm_w = np.asarray(inputs["norm_w"], np.float32)
    norm_b = np.asarray(inputs["norm_b"], np.float32)

    def split_hl(a, dtyp):
        hi = a.astype(dtyp)
        lo = (a - hi.astype(np.float32)).astype(dtyp)
        return hi, lo

    w1T = np.ascontiguousarray(fc1_w.T)                      # [f, h]
    w2T = np.ascontiguousarray(fc2_w.T)                      # [h, o]
    featT = np.ascontiguousarray(feature.reshape(B * C, FF).T)   # [f, bc]
    fth, ftl = split_hl(featT, np.float16)
    # [128, 72, 2, 48]
    featT_b = np.ascontiguousarray(
        np.stack([fth, ftl], axis=1).reshape(KT1, P, 2, B * C)
        .transpose(1, 0, 2, 3))
    ls = np.exp(np.minimum(logit_scale.reshape(HW), np.log(np.float32(100.0))))
    ls_b = np.ascontiguousarray(ls.reshape(MBS, P).T).astype(np.float32)
    ones1 = np.ones((1, B * C), np.float32)
    id48 = np.eye(48, dtype=np.float32)
    blk = np.zeros((48, NT), np.float32)
    blk[np.arange(48), np.arange(48) // C] = 1.0
    blkT = np.ascontiguousarray(blk.T)
    wb48 = np.ascontiguousarray(
        np.stack([np.tile(norm_w, NT), np.tile(norm_b, NT)], axis=1))
    b2 = (fc2_b / NCORES).reshape(1, HW).astype(np.float32)

    in_maps = []
    for k in range(NCORES):
        w1k = w1T[:, k * HS:(k + 1) * HS].astype(np.float16)   # [9216, 1152]
        # [24, 128, 3, 1152]
        w1s = np.ascontiguousarray(
            w1k.reshape(NC1, CH1, P, HS).transpose(0, 2, 1, 3))
        b1k = np.ascontiguousarray(fc1_b[k * HS:(k + 1) * HS]).reshape(1, HS)
        w2k = w2T[k * HS:(k + 1) * HS, :].astype(np.float16)   # [1152, 4096]
        # [4, 128, 9, 1024]
        w2s = np.ascontiguousarray(
            w2k.reshape(JT, P, NGRP, 1024).transpose(2, 1, 0, 3))
        xh, xl = split_hl(x[k], bf16)                          # [6, 4096]
        xaug = np.concatenate([xh, xl, xh, xl], axis=0)        # [24, 4096]
        xtbk = np.ascontiguousarray(x[k].T.reshape(MBS, P, C).transpose(1, 0, 2))
        xmax_k = np.linalg.norm(x[k], axis=0).max()
        # 2^63.5 rescale of the bitcast-sqrt seed folded in
        negx = np.full((P, 1), -xmax_k * 2.0**63.5, np.float32)
        srt = np.ascontiguousarray(
            np.broadcast_to(np.array([[1, 0x1FBD1DF5]], np.int32), (P, 2)))
        in_maps.append({
            "featT": featT_b, "w1t": w1s, "b1": b1k, "w2t": w2s, "b2": b2,
            "xaug": xaug, "xtb": xtbk, "lsb": ls_b, "ones1": ones1,
            "id48": id48, "blk": blk, "blkT": blkT, "wb48": wb48,
            "negx": negx, "srtc": srt,
        })
    return in_maps


def _install_ntff_shim():
    # The agent image's `antenv` lacks `axon_hooks`, which bass_utils needs
    # for trace=True under axon. Fabricate the registry module and install
    # the ctypes-based NTFF hook against libaxon_pjrt.so.
    import sys
    import types
    import ctypes
    import contextlib

    try:
        import antenv.axon_hooks  # noqa: F401
        return
    except ImportError:
        pass
    if "antenv.axon_hooks" in sys.modules:
        return
    mod = types.ModuleType("antenv.axon_hooks")
    _h = [None]
    mod.set_axon_ntff_profile_hook = lambda h: _h.__setitem__(0, h)
    mod.get_axon_ntff_profile_hook = lambda: _h[0]
    sys.modules["antenv.axon_hooks"] = mod

    so_path = "/opt/axon/libaxon_pjrt.so"
    if not os.path.exists(so_path):
        return
    lib = ctypes.CDLL(so_path)
    if not hasattr(lib, "axon_start_nrt_profile"):
        return
    lib.axon_start_nrt_profile.argtypes = [
        ctypes.POINTER(ctypes.c_int64), ctypes.c_size_t]
    lib.axon_start_nrt_profile.restype = ctypes.c_int64
    lib.axon_stop_nrt_profile.argtypes = [ctypes.c_char_p]
    lib.axon_stop_nrt_profile.restype = ctypes.c_int64

    @contextlib.contextmanager
    def _hook(output_dir, device_ids):
        import jax
        jax.devices()
        if device_ids:
            ids = (ctypes.c_int64 * len(device_ids))(*device_ids)
            rc = lib.axon_start_nrt_profile(ids, len(device_ids))
        else:
            rc = lib.axon_start_nrt_profile(None, 0)
        if rc != 0:
            raise RuntimeError(f"axon_start_nrt_profile rc={rc}")
        try:
            yield
        finally:
            n = lib.axon_stop_nrt_profile(str(output_dir).encode())
            print(f"ntff profile: {n} file(s) written to {output_dir}")

    mod.set_axon_ntff_profile_hook(_hook)


def kernel(**inputs):
    from concourse.bass_utils import run_bass_kernel_spmd

    if bool(int(os.environ.get("BASS_KT_TRACE", "0"))):
        _install_ntff_shim()

    if "nc" not in _cache:
        _cache["nc"] = _build_program()
    nc = _cache["nc"]

    in_maps = _host_prep(inputs)
    trace = bool(int(os.environ.get("BASS_KT_TRACE", "0")))
    res = run_bass_kernel_spmd(nc, in_maps, list(range(NCORES)), trace=trace)
    kernel.last_results = res
    out = np.stack([np.asarray(res.results[k]["out"]) for k in range(NCORES)])
    return out.astype(np.float32)


# revision 30
# speedup vs baseline: 1.3572x; 1.0595x over previous
# kernel.py — Trainium2 Bass kernel for nn_ChannelAttentionBlock (v3)
#
# Computation (per reference):
#   h = relu(feature @ fc1_w.T + fc1_b)            [B,C,FF]
#   f = h @ fc2_w.T + fc2_b                        [B,C,HW]
#   T[b,n,m] = sum_c x[b,c,n] * f[b,c,m] * ls[m]   (ls = exp(min(logit_scale, log 100)))
#   P = softmax_n(T);  out[b,n,c] = sum_m P[n,m] x[b,c,m];  LayerNorm over c; -> [B,C,HW]
#
# Sharding (8 cores):
#   MLP tensor-parallel: fc1 split on hidden (each core 1152 of 9216 hidden),
#   fc2 split on input; ReduceScatter over batch so core k receives f[b=k].
#   Attention data-parallel: core k handles batch k entirely.
#
# Precision (PE streams 1 col/cyc for bf16/fp16, fp32r is 1 cyc when N>=256):
#   MLP1/MLP2: fp16 weights single-plane + fp16 hi/lo activations, 2 pairings
#     (fh.w + fl.w) — measured 7e-3 rel err vs the 2e-2 budget. Halves w1 DMA
#     (21.2MB/core) and cuts MLP PE work by a third vs bf16 3-way.
#   mm1 (T = f^T x, K=24): bf16 hi/lo K-stacked — full precision, free in K.
#   mm2 (out += xp^T E): fp32r on fp32 et (1cyc/row at N=512).
#   Softmax shift: per-m Cauchy-Schwarz bound -ls*xmax*||f[:,m]||+40.
#
# Schedule (v3): phase-separated for PE clock-gate hygiene. The PE drops to
# 1.2 GHz after a >3.4us idle and only re-promotes after ~3us of gapless
# execution, so work is organized in long wait-free streaks:
#   [w1 DMA || MLP1] -> [MLP2 all 4 groups back-to-back; RS(g) issued as each
#   group finishes, RS wait hidden under later groups] -> [attention: 32
#   m-block slots; 2048-wide exp on 2x[128,4,512] PSUM rotation (all 8 banks);
#   mm2 of the lagged pair writes into exp-drained sub-banks of the previous
#   tile; per-slot PE burst ~3.5us vs ACT 4.2us] -> LayerNorm.

import os
import numpy as np

B, C, HW, FF, P = 8, 6, 4096, 9216, 128
NCORES = 8
HS = FF // NCORES        # 1152
KT1 = FF // P            # 72 fc1 K tiles
CH1 = 3                  # fc1 K tiles per DMA chunk
NC1 = KT1 // CH1         # 24 fc1 chunks
JT = HS // P             # 9  fc2 K tiles
NT = HW // 512           # 8  512-wide n/o chunks
MBS = HW // P            # 32 m blocks
NGRP = 4                 # reduce-scatter groups (2 o-chunks each)
MB_PER_G = MBS // NGRP   # 8
EPS = 1e-5
SHIFT_HEADROOM = 40.0

_cache = {}


def _build_program():
    import concourse.bacc as bacc
    import concourse.bass as bass
    import concourse.tile as tile
    import concourse.mybir as mybir

    dt = mybir.dt.float32
    dtr = mybir.dt.float32r
    dtb = mybir.dt.bfloat16
    dth = mybir.dt.float16
    AF = mybir.ActivationFunctionType
    ALU = mybir.AluOpType
    AX = mybir.AxisListType

    nc = bacc.Bacc(
        "TRN2",
        target_bir_lowering=False,
        debug=False,
        enable_asserts=False,
        num_devices=NCORES,
    )

    # ---- external I/O ----
    featT_d = nc.dram_tensor("featT", [3, P, KT1 // 3, 2, 48], dth,
                             kind="ExternalInput").ap()
    w1_d = nc.dram_tensor("w1t", [NC1, P, CH1, HS], dth, kind="ExternalInput").ap()
    b1_d = nc.dram_tensor("b1", [1, HS], dt, kind="ExternalInput").ap()
    w2_d = nc.dram_tensor("w2t", [NGRP, P, JT, 1024], dth, kind="ExternalInput").ap()
    b2_d = nc.dram_tensor("b2", [1, HW], dt, kind="ExternalInput").ap()  # fc2_b/8
    xaug_d = nc.dram_tensor("xaug", [24, HW], dtb, kind="ExternalInput").ap()
    xtb_d = nc.dram_tensor("xtb", [P, MBS, C], dt, kind="ExternalInput").ap()
    ls_d = nc.dram_tensor("lsb", [P, MBS], dt, kind="ExternalInput").ap()
    srt_d = nc.dram_tensor("srtc", [P, 2], mybir.dt.int32,
                           kind="ExternalInput").ap()
    ones_d = nc.dram_tensor("ones1", [1, 48], dt, kind="ExternalInput").ap()
    id48_d = nc.dram_tensor("id48", [48, 48], dt, kind="ExternalInput").ap()
    blk_d = nc.dram_tensor("blk", [48, NT], dt, kind="ExternalInput").ap()
    blkT_d = nc.dram_tensor("blkT", [NT, 48], dt, kind="ExternalInput").ap()
    wb48_d = nc.dram_tensor("wb48", [48, 2], dt, kind="ExternalInput").ap()
    negx_d = nc.dram_tensor("negx", [P, 1], dt, kind="ExternalInput").ap()
    out_d = nc.dram_tensor("out", [C, HW], dt, kind="ExternalOutput").ap()

    with tile.TileContext(nc) as tc:
        # float32r APs carry full-fp32 bit patterns; the PE rounds at load.
        with nc.allow_low_precision(reason="fp32r/fp16/bf16 kernel dataflow"), \
             tc.tile_pool(name="const", bufs=1) as const, \
             tc.tile_pool(name="dram", bufs=1, space="DRAM") as dram:

            # ---- constants / small inputs ----
            xaug_sb = const.tile([24, HW], dtb, tag="xaug")
            nc.gpsimd.dma_start(out=xaug_sb[:], in_=xaug_d)
            xtb_sb = const.tile([P, MBS, C], dt, tag="xtb")
            nc.gpsimd.dma_start(out=xtb_sb[:], in_=xtb_d)
            ls_sb = const.tile([P, MBS], dt, tag="ls")
            nc.gpsimd.dma_start(out=ls_sb[:], in_=ls_d)
            ones_sb = const.tile([1, 48], dtr, tag="ones1")
            nc.gpsimd.dma_start(out=ones_sb[:], in_=ones_d.bitcast(dtr))
            id48_sb = const.tile([48, 48], dt, tag="id48")
            nc.gpsimd.dma_start(out=id48_sb[:], in_=id48_d)
            negx_sb = const.tile([P, 1], dt, tag="negx")
            nc.gpsimd.dma_start(out=negx_sb[:], in_=negx_d)
            srt_sb = const.tile([P, 2], mybir.dt.int32, tag="srtc")
            nc.gpsimd.dma_start(out=srt_sb[:], in_=srt_d)
            blk_sb = const.tile([48, NT], dtr, tag="blk")
            nc.gpsimd.dma_start(out=blk_sb[:], in_=blk_d.bitcast(dtr))
            blkT_sb = const.tile([NT, 48], dtr, tag="blkT")
            nc.gpsimd.dma_start(out=blkT_sb[:], in_=blkT_d.bitcast(dtr))
            wb48_sb = const.tile([48, 2], dt, tag="wb48")
            nc.gpsimd.dma_start(out=wb48_sb[:], in_=wb48_d)

            # h^T in fp16 hi/lo planes (filled after MLP1)
            hT_sb = const.tile([P, JT, 2, 48], dth, tag="hT")
            faug_sb = const.tile([24, HW], dtb, tag="faug")
            # per-m softmax shift (exp bias): -ls*xmax*||f[:,m]|| + 40
            shift_sb = const.tile([P, MBS], dt, tag="shift")
            zero128_sb = const.tile([P, 1], dt, tag="zero128")
            nc.vector.memset(zero128_sb[:], 0.0)
            ones6_sb = const.tile([C, 1], dtb, tag="ones6")
            nc.vector.memset(ones6_sb[:], 1.0)
            # attention output accumulators, one [6,512] tile per n-chunk
            O_nt = [const.tile([C, 512], dt, tag=f"O{nt}", name=f"O{nt}")
                    for nt in range(NT)]
            wz_sb = const.tile([1, 640], dt, tag="wz")
            nc.vector.memset(wz_sb[:], 0.0)
            O48_sb = const.tile([48, 512], dt, tag="O48")

            # DRAM bounce buffers for the reduce-scatter groups
            rs_in = [dram.tile([48, 2, 512], dt, tag=f"rsin{g}", name=f"rsin{g}")
                     for g in range(NGRP)]
            rs_out = [dram.tile([C, 2, 512], dt, tag=f"rsout{g}", name=f"rsout{g}")
                      for g in range(NGRP)]
            wrm_in = dram.tile([1, 8], dt, tag="wrmin", name="wrmin")
            wrm_out = dram.tile([1, 8], dt, tag="wrmout", name="wrmout")

            # ============ warmup: PE clock promote, exp tables, CC cores ============
            with tc.tile_pool(name="wrm", bufs=1, space="PSUM") as wrmp:
                wps = wrmp.tile([P, 512], dt, tag="wps")
                for _ in range(12):
                    nc.tensor.matmul(
                        wps[:],
                        lhsT=wz_sb[:, 0:P].bitcast(dtr),
                        rhs=wz_sb[:, P:P + 512].bitcast(dtr),
                        start=True,
                        stop=True,
                    )
                # load the exp table set now (Exp+Relu share it; the kernel
                # body uses no other table until the final LayerNorm)
                wex = wrmp.tile([1, 8], dt, tag="wex")
                nc.scalar.activation(wex[:], wz_sb[:, 0:8], AF.Exp,
                                     bias=zero128_sb[0:1, :])
            # init + warm the collective path (RDH state, cross-core skew)
            nc.gpsimd.dma_start(out=wrm_in[:], in_=wz_sb[:, 0:8])
            nc.gpsimd.collective_compute(
                "AllReduce",
                ALU.add,
                replica_groups=[list(range(NCORES))],
                ins=[wrm_in.opt()],
                outs=[wrm_out.opt()],
            )

            # ====== MLP1 (w2 DMAs interleaved) / MLP2 + RS / fprep(0) ======
            with tc.tile_pool(name="w2p", bufs=NGRP) as w2p, \
                 tc.tile_pool(name="m2c", bufs=1) as m2c, \
                 tc.tile_pool(name="fts", bufs=2) as ftsp, \
                 tc.tile_pool(name="fps", bufs=2, space="PSUM") as fps, \
                 tc.tile_pool(name="fpe", bufs=2) as fpep:

                def emit_fprep(g, get_ptile):
                    # runs once RS(g) lands: distribute f, build faug + shift.
                    # get_ptile() -> [P, 512] PSUM AP for the norm matmuls.
                    gsl = slice(g * 1024, (g + 1) * 1024)
                    fgrp = ftsp.tile([C, 1024], dt, tag="fgrp")
                    nc.sync.dma_start(out=fgrp[:], in_=rs_out[g][:])
                    # bf16 hi/lo split of f for mm1, rows [fh,fh,fl,fl]
                    fh = ftsp.tile([C, 1024], dtb, tag="fh")
                    nc.vector.tensor_copy(fh[:], fgrp[:])
                    fl = ftsp.tile([C, 1024], dtb, tag="fl")
                    nc.vector.tensor_sub(fl[:], fgrp[:], fh[:])
                    nc.sync.dma_start(out=faug_sb[0:C, gsl], in_=fh[:])
                    nc.sync.dma_start(out=faug_sb[C:2 * C, gsl], in_=fh[:])
                    nc.sync.dma_start(out=faug_sb[2 * C:3 * C, gsl], in_=fl[:])
                    nc.sync.dma_start(out=faug_sb[3 * C:24, gsl], in_=fl[:])
                    # shift[m] = -ls[m]*xmax*||f[:,m]||_2 + 40; column sq-norms
                    # via tiny matmuls into a borrowed PSUM tile, then sqrt on
                    # DVE via the bitcast seed (+-3.5%, harmless in a bound
                    # with ~47 e-folds of margin). No ACT => no table reloads.
                    fsq = ftsp.tile([C, 1024], dtb, tag="fsq")
                    nc.vector.tensor_mul(fsq[:], fgrp[:], fgrp[:])
                    pt = get_ptile()
                    for chn in range(NT):
                        nc.tensor.matmul(
                            pt[:, chn:chn + 1],
                            lhsT=fsq[:, chn * P:(chn + 1) * P],
                            rhs=ones6_sb[:],
                            start=True,
                            stop=True,
                        )
                    # y = bitcast(i >> 1) = sqrt(S) * 2^-63.5 * (1..1.061);
                    # the 2^63.5 rescale is folded into negx on the host
                    fns = ftsp.tile([P, NT], mybir.dt.int32, tag="fns")
                    nc.vector.tensor_scalar(
                        out=fns[:], in0=pt[:, 0:NT].bitcast(mybir.dt.int32),
                        scalar1=srt_sb[:, 0:1], scalar2=0.0,
                        op0=ALU.arith_shift_right, op1=ALU.bypass,
                    )
                    fnl = ftsp.tile([P, NT], dt, tag="fnl")
                    nc.vector.tensor_mul(
                        fnl[:], fns[:].bitcast(dt),
                        ls_sb[:, NT * g:NT * (g + 1)])
                    nc.vector.tensor_scalar(
                        out=shift_sb[:, NT * g:NT * (g + 1)], in0=fnl[:],
                        scalar1=negx_sb[:], scalar2=SHIFT_HEADROOM,
                        op0=ALU.mult, op1=ALU.add,
                    )

                b2_sb = m2c.tile([1, HW], dtr, tag="b2")
                nc.gpsimd.dma_start(out=b2_sb[:], in_=b2_d.bitcast(dtr))
                w2t = {}

                def fetch_w2(g, token=None):
                    # `token`: 1-elem copy into the tile first, so the DMA
                    # inherits a dependency and the scheduler cannot hoist it
                    # to t=0 where it would starve the MLP1 weight stream
                    w2t[g] = w2p.tile([P, JT, 1024], dth, tag="w2t",
                                      name=f"w2t{g}")
                    if token is not None:
                        nc.vector.tensor_copy(w2t[g][0:1, 0, 0:1], token)
                    nc.sync.dma_start(out=w2t[g][:], in_=w2_d[g])



                # ---- MLP1: h = relu(feat @ w1 + b1), fp16 2-way ----
                with tc.tile_pool(name="w1p", bufs=4) as w1p, \
                     tc.tile_pool(name="m1c", bufs=1) as m1c, \
                     tc.tile_pool(name="ps1", bufs=1, space="PSUM") as ps1, \
                     tc.tile_pool(name="pst", bufs=2, space="PSUM") as pst:
                    featT_sb = m1c.tile([P, KT1, 2, 48], dth, tag="featT")
                    for pc in range(3):  # split: piece 0 unblocks MLP1 early
                        nc.gpsimd.dma_start(
                            out=featT_sb[:, 24 * pc:24 * (pc + 1), :, :],
                            in_=featT_d[pc])
                    b1_sb = m1c.tile([1, HS], dtr, tag="b1")
                    nc.gpsimd.dma_start(out=b1_sb[:], in_=b1_d.bitcast(dtr))
                    h_sb = m1c.tile([48, HS], dt, tag="h")
                    hp = ps1.tile([48, 3, 512], dt, tag="hp")
                    for ch in range(NC1):
                        w1t = w1p.tile([P, CH1, HS], dth, tag="w1t")
                        nc.sync.dma_start(out=w1t[:], in_=w1_d[ch])
                        if ch == 15:
                            # w2 g0 starts once chunk 15 has landed: early
                            # enough to be resident for MLP2, late enough not
                            # to starve the w1 stream
                            fetch_w2(0, token=w1t[0:1, 0, 0:1])
                        for s in range(CH1):
                            kk = CH1 * ch + s
                            for hl in range(2):
                                lhs = featT_sb[:, kk, hl, :]
                                for j in range(3):
                                    nc.tensor.matmul(
                                        hp[:, j, 0:384],
                                        lhsT=lhs,
                                        rhs=w1t[:, s, j * 384:(j + 1) * 384],
                                        start=(kk == 0 and hl == 0),
                                        stop=False,
                                    )
                    for j in range(3):  # bias via K=1 ones row (fp32r)
                        nc.tensor.matmul(
                            hp[:, j, 0:384],
                            lhsT=ones_sb[:],
                            rhs=b1_sb[:, j * 384:(j + 1) * 384],
                            start=False,
                            stop=True,
                        )
                    for j in range(3):  # relu eviction on ACT
                        nc.scalar.activation(
                            h_sb[:, j * 384:(j + 1) * 384],
                            hp[:, j, 0:384],
                            AF.Relu,
                        )
                    # transpose h -> hT (9 PE transposes), split fp16 hi/lo
                    for t in range(JT):
                        tp = pst.tile([P, 48], dt, tag="tp")
                        nc.tensor.transpose(
                            tp[:], h_sb[:, t * P:(t + 1) * P], id48_sb[:]
                        )
                        nc.vector.tensor_copy(hT_sb[:, t, 0, :], tp[:])
                        nc.vector.tensor_sub(hT_sb[:, t, 1, :], tp[:],
                                             hT_sb[:, t, 0, :])

                # ---- MLP2 groups + RS; fprep(0) tucked after group 1 so its
                # faug/shift are ready the moment the PE finishes group 3 ----
                def emit_mlp2_group(g):
                    for occ in range(2):
                        oc = 2 * g + occ
                        fp = fps.tile([P, 512], dt, tag="fp")
                        for jj in range(JT):
                            for hl in range(2):
                                nc.tensor.matmul(
                                    fp[0:48, :],
                                    lhsT=hT_sb[:, jj, hl, :],
                                    rhs=w2t[g][:, jj, occ * 512:(occ + 1) * 512],
                                    start=(jj == 0 and hl == 0),
                                    stop=False,
                                )
                        nc.tensor.matmul(  # + fc2_b/8 (summed to fc2_b by RS)
                            fp[0:48, :],
                            lhsT=ones_sb[:],
                            rhs=b2_sb[:, oc * 512:(oc + 1) * 512],
                            start=False,
                            stop=True,
                        )
                        fpe = fpep.tile([48, 512], dt, tag="fpe")
                        nc.vector.tensor_copy(fpe[:], fp[0:48, :])
                        nc.sync.dma_start(out=rs_in[g][:, occ, :], in_=fpe[:])
                        if occ == 1:
                            nc.gpsimd.collective_compute(
                                "ReduceScatter",
                                ALU.add,
                                replica_groups=[list(range(NCORES))],
                                ins=[rs_in[g].opt()],
                                outs=[rs_out[g].opt()],
                            )

                fetch_w2(1, token=hT_sb[0:1, JT - 1, 1, 0:1])
                emit_mlp2_group(0)
                fetch_w2(2)       # reuses g0's buffer once its reads retire
                emit_mlp2_group(1)
                fetch_w2(3)
                emit_mlp2_group(2)
                emit_mlp2_group(3)
                # fprep(0) after all MLP2 matmuls: its tiny norm-matmuls gate
                # on RS(0), and nothing else may sit behind them in PE order
                emit_fprep(0, lambda: fps.tile([P, 512], dt, tag="fp",
                                               name="fprep0ps")[:])

            # ===================== attention =====================
            with tc.tile_pool(name="tpp", bufs=2, space="PSUM") as tpp, \
                 tc.tile_pool(name="fts2", bufs=2) as ftsp2, \
                 tc.tile_pool(name="etp", bufs=5) as etp, \
                 tc.tile_pool(name="xpp", bufs=5) as xpp, \
                 tc.tile_pool(name="accp", bufs=4) as accp:

                first_flush = [True] * NT
                ets = {}        # mb -> (et tile, xp tile)
                prev_tps = [None]
                ftsp = ftsp2  # fprep called below uses the attention pool

                def emit_mm2_chunk(pmb, ntc, dst):
                    # one n-chunk of pair (pmb, pmb+1): 2 accumulating MMs into
                    # an exp-drained PSUM sub-bank, then DVE-accumulate into O
                    for q in range(2):
                        et_q, xp_q = ets[pmb + q]
                        nc.tensor.matmul(
                            dst,
                            lhsT=xp_q[:].bitcast(dtr),
                            rhs=et_q[:, ntc, :].bitcast(dtr),
                            start=(q == 0),
                            stop=(q == 1),
                        )
                    if first_flush[ntc]:
                        nc.vector.tensor_copy(O_nt[ntc][:].bitcast(dtr), dst)
                        first_flush[ntc] = False
                    else:
                        nc.vector.tensor_add(
                            O_nt[ntc][:].bitcast(dtr), O_nt[ntc][:], dst)
                    if ntc == NT - 1:
                        del ets[pmb]
                        del ets[pmb + 1]

                def emit_slot(mb):
                    # one m-block: 2 x (2048-wide mm1 + exp); mm2 of the pair
                    # lagged by 2 m-blocks targets the previous (drained) tile
                    et = etp.tile([P, NT, 512], dt, tag="et")
                    acc = accp.tile([P, 2], dt, tag="acc")
                    lhs = faug_sb[:, mb * P:(mb + 1) * P]
                    for t in range(2):
                        tps = tpp.tile([P, 4, 512], dt, tag="tps")
                        for i in range(4):
                            nt_ = 4 * t + i
                            nc.tensor.matmul(
                                tps[:, i, :],
                                lhsT=lhs,
                                rhs=xaug_sb[:, nt_ * 512:(nt_ + 1) * 512],
                                start=True,
                                stop=True,
                            )
                        nc.scalar.activation(
                            et[:, 4 * t:4 * t + 4, :].bitcast(dtr),
                            tps[:],
                            AF.Exp,
                            scale=ls_sb[:, mb:mb + 1],
                            bias=shift_sb[:, mb:mb + 1],
                            accum_out=acc[:, t:t + 1],
                        )
                        if mb >= 2:
                            pmb = (mb // 2 - 1) * 2
                            base = (mb % 2) * 4 + 2 * t
                            for i2 in range(2):
                                emit_mm2_chunk(
                                    pmb, base + i2,
                                    prev_tps[0][0:C, 2 * t + i2, :])
                        prev_tps[0] = tps
                    cs = accp.tile([P, 1], dt, tag="cs")
                    nc.vector.reduce_sum(cs[:], acc[:], AX.X)
                    rc = accp.tile([P, 1], dt, tag="rc")
                    nc.vector.reciprocal(rc[:], cs[:])
                    xp = xpp.tile([P, C], dt, tag="xp")
                    nc.vector.tensor_scalar_mul(
                        xp[:].bitcast(dtr), xtb_sb[:, mb, :], rc[:])
                    ets[mb] = (et, xp)

                def tpp_ptile():
                    t = tpp.tile([P, 4, 512], dt, tag="tps", name="tpsx")
                    return t[:, 0, :]

                # fprep(0) already ran during MLP2; prep group g+1 one group
                # early, two slots in (so its RS-gated matmuls sit behind
                # already-runnable slot work in the PE stream)
                for g in range(NGRP):
                    for mb in range(g * MB_PER_G, (g + 1) * MB_PER_G):
                        emit_slot(mb)
                        if mb == g * MB_PER_G + 1 and g + 1 < NGRP:
                            emit_fprep(g + 1, tpp_ptile)
                # drain: last pair's mm2 chunks
                for d in range(2):
                    tps = tpp.tile([P, 4, 512], dt, tag="tps")
                    for i in range(4):
                        emit_mm2_chunk(MBS - 2, 4 * d + i, tps[0:C, i, :])

                # stack the 8 [6,512] chunks into [48,512] (partition-offset
                # SBUF->SBUF DMAs)
                for nt_ in range(NT):
                    nc.sync.dma_start(
                        out=O48_sb[C * nt_:C * nt_ + C, :].bitcast(dtr),
                        in_=O_nt[nt_][:].bitcast(dtr))

            # ===================== LayerNorm over c + output =====================
            with tc.tile_pool(name="lnps", bufs=2, space="PSUM") as lnps, \
                 tc.tile_pool(name="lnrp", bufs=2, space="PSUM") as lnrp, \
                 tc.tile_pool(name="lnsb", bufs=1) as lnsb:
                eps_sb = lnsb.tile([NT, 1], dt, tag="eps")
                nc.vector.memset(eps_sb[:], EPS)
                O2_sb = lnsb.tile([48, 512], dt, tag="O2")
                nc.vector.tensor_mul(O2_sb[:].bitcast(dtr), O48_sb[:], O48_sb[:])
                s_ps = lnps.tile([NT, 512], dt, tag="sps")
                nc.tensor.matmul(
                    s_ps[:], lhsT=blk_sb[:], rhs=O48_sb[:].bitcast(dtr),
                    start=True, stop=True,
                )
                s2_ps = lnps.tile([NT, 512], dt, tag="s2ps")
                nc.tensor.matmul(
                    s2_ps[:], lhsT=blk_sb[:], rhs=O2_sb[:].bitcast(dtr),
                    start=True, stop=True,
                )
                mean_sb = lnsb.tile([NT, 512], dt, tag="mean")
                nc.vector.tensor_scalar_mul(
                    mean_sb[:].bitcast(dtr), s_ps[:], 1.0 / C)
                ms_sb = lnsb.tile([NT, 512], dt, tag="ms")
                nc.vector.tensor_mul(ms_sb[:], mean_sb[:], mean_sb[:])
                var_sb = lnsb.tile([NT, 512], dt, tag="var")
                nc.vector.tensor_scalar_mul(var_sb[:], s2_ps[:], 1.0 / C)
                nc.vector.tensor_sub(var_sb[:], var_sb[:], ms_sb[:])
                # 1/sqrt(var+eps) via ln/exp (same resident table set)
                vln_sb = lnsb.tile([NT, 512], dt, tag="vln")
                nc.scalar.activation(vln_sb[:], var_sb[:], AF.Ln, bias=eps_sb[:])
                rstd_sb = lnsb.tile([NT, 512], dt, tag="rstd")
                nc.scalar.activation(rstd_sb[:].bitcast(dtr), vln_sb[:], AF.Exp,
                                     scale=-0.5, bias=zero128_sb[0:NT, :])
                mrep = lnrp.tile([48, 512], dt, tag="mrep")
                nc.tensor.matmul(
                    mrep[:], lhsT=blkT_sb[:], rhs=mean_sb[:].bitcast(dtr),
                    start=True, stop=True,
                )
                rrep = lnrp.tile([48, 512], dt, tag="rrep")
                nc.tensor.matmul(
                    rrep[:], lhsT=blkT_sb[:], rhs=rstd_sb[:].bitcast(dtr),
                    start=True, stop=True,
                )
                on_sb = lnsb.tile([48, 512], dt, tag="on")
                nc.vector.tensor_sub(on_sb[:], O48_sb[:], mrep[:])
                nc.vector.tensor_mul(on_sb[:], on_sb[:], rrep[:])
                nc.vector.tensor_scalar(
                    out=on_sb[:], in0=on_sb[:],
                    scalar1=wb48_sb[:, 0:1], scalar2=wb48_sb[:, 1:2],
                    op0=ALU.mult, op1=ALU.add,
                )
                for nt_ in range(NT):
                    nc.sync.dma_start(
                        out=out_d[:, nt_ * 512:(nt_ + 1) * 512],
                        in_=on_sb[C * nt_:C * nt_ + C, :],
                    )

    nc.compile()
    return nc


def _host_prep(inputs):
    import ml_dtypes
    bf16 = ml_dtypes.bfloat16

    x = np.asarray(inputs["x"], np.float32)
    feature = np.asarray(inputs["feature"], np.float32)
    fc1_w = np.asarray(inputs["fc1_w"], np.float32)
    fc1_b = np.asarray(inputs["fc1_b"], np.float32)
    fc2_w = np.asarray(inputs["fc2_w"], np.float32)
    fc2_b = np.asarray(inputs["fc2_b"], np.float32)
    logit_scale = np.asarray(inputs["logit_scale"], np.float32)
    norm_w = np.asarray(inputs["norm_w"], np.float32)
    norm_b = np.asarray(inputs["norm_b"], np.float32)

    def split_hl(a, dtyp):
        hi = a.astype(dtyp)
        lo = (a - hi.astype(np.float32)).astype(dtyp)
        return hi, lo

    w1T = np.ascontiguousarray(fc1_w.T)                      # [f, h]
    w2T = np.ascontiguousarray(fc2_w.T)                      # [h, o]
    featT = np.ascontiguousarray(feature.reshape(B * C, FF).T)   # [f, bc]
    fth, ftl = split_hl(featT, np.float16)
    # [3, 128, 24, 2, 48] — three k-range pieces for split DMA
    featT_b = np.ascontiguousarray(
        np.stack([fth, ftl], axis=1).reshape(KT1, P, 2, B * C)
        .transpose(1, 0, 2, 3).reshape(P, 3, KT1 // 3, 2, B * C)
        .transpose(1, 0, 2, 3, 4))
    ls = np.exp(np.minimum(logit_scale.reshape(HW), np.log(np.float32(100.0))))
    ls_b = np.ascontiguousarray(ls.reshape(MBS, P).T).astype(np.float32)
    ones1 = np.ones((1, B * C), np.float32)
    id48 = np.eye(48, dtype=np.float32)
    blk = np.zeros((48, NT), np.float32)
    blk[np.arange(48), np.arange(48) // C] = 1.0
    blkT = np.ascontiguousarray(blk.T)
    wb48 = np.ascontiguousarray(
        np.stack([np.tile(norm_w, NT), np.tile(norm_b, NT)], axis=1))
    b2 = (fc2_b / NCORES).reshape(1, HW).astype(np.float32)

    in_maps = []
    for k in range(NCORES):
        w1k = w1T[:, k * HS:(k + 1) * HS].astype(np.float16)   # [9216, 1152]
        # [24, 128, 3, 1152]
        w1s = np.ascontiguousarray(
            w1k.reshape(NC1, CH1, P, HS).transpose(0, 2, 1, 3))
        b1k = np.ascontiguousarray(fc1_b[k * HS:(k + 1) * HS]).reshape(1, HS)
        w2k = w2T[k * HS:(k + 1) * HS, :].astype(np.float16)   # [1152, 4096]
        # [4, 128, 9, 1024]
        w2s = np.ascontiguousarray(
            w2k.reshape(JT, P, NGRP, 1024).transpose(2, 1, 0, 3))
        xh, xl = split_hl(x[k], bf16)                          # [6, 4096]
        xaug = np.concatenate([xh, xl, xh, xl], axis=0)        # [24, 4096]
        xtbk = np.ascontiguousarray(x[k].T.reshape(MBS, P, C).transpose(1, 0, 2))
        xmax_k = np.linalg.norm(x[k], axis=0).max()
        # 2^63.5 rescale of the bitcast-sqrt seed folded in
        negx = np.full((P, 1), -xmax_k * 2.0**63.5, np.float32)
        srt = np.ascontiguousarray(
            np.broadcast_to(np.array([[1, 0x1FBD1DF5]], np.int32), (P, 2)))
        in_maps.append({
            "featT": featT_b, "w1t": w1s, "b1": b1k, "w2t": w2s, "b2": b2,
            "xaug": xaug, "xtb": xtbk, "lsb": ls_b, "ones1": ones1,
            "id48": id48, "blk": blk, "blkT": blkT, "wb48": wb48,
            "negx": negx, "srtc": srt,
        })
    return in_maps


def _install_ntff_shim():
    # The agent image's `antenv` lacks `axon_hooks`, which bass_utils needs
    # for trace=True under axon. Fabricate the registry module and install
    # the ctypes-based NTFF hook against libaxon_pjrt.so.
    import sys
    import types
    import ctypes
    import contextlib

    try:
        import antenv.axon_hooks  # noqa: F401
        return
    except ImportError:
        pass
    if "antenv.axon_hooks" in sys.modules:
        return
    mod = types.ModuleType("antenv.axon_hooks")
    _h = [None]
    mod.set_axon_ntff_profile_hook = lambda h: _h.__setitem__(0, h)
    mod.get_axon_ntff_profile_hook = lambda: _h[0]
    sys.modules["antenv.axon_hooks"] = mod

    so_path = "/opt/axon/libaxon_pjrt.so"
    if not os.path.exists(so_path):
        return
    lib = ctypes.CDLL(so_path)
    if not hasattr(lib, "axon_start_nrt_profile"):
        return
    lib.axon_start_nrt_profile.argtypes = [
        ctypes.POINTER(ctypes.c_int64), ctypes.c_size_t]
    lib.axon_start_nrt_profile.restype = ctypes.c_int64
    lib.axon_stop_nrt_profile.argtypes = [ctypes.c_char_p]
    lib.axon_stop_nrt_profile.restype = ctypes.c_int64

    @contextlib.contextmanager
    def _hook(output_dir, device_ids):
        import jax
        jax.devices()
        if device_ids:
            ids = (ctypes.c_int64 * len(device_ids))(*device_ids)
            rc = lib.axon_start_nrt_profile(ids, len(device_ids))
        else:
            rc = lib.axon_start_nrt_profile(None, 0)
        if rc != 0:
            raise RuntimeError(f"axon_start_nrt_profile rc={rc}")
        try:
            yield
        finally:
            n = lib.axon_stop_nrt_profile(str(output_dir).encode())
            print(f"ntff profile: {n} file(s) written to {output_dir}")

    mod.set_axon_ntff_profile_hook(_hook)


def kernel(**inputs):
    from concourse.bass_utils import run_bass_kernel_spmd

    if bool(int(os.environ.get("BASS_KT_TRACE", "0"))):
        _install_ntff_shim()

    if "nc" not in _cache:
        _cache["nc"] = _build_program()
    nc = _cache["nc"]

    in_maps = _host_prep(inputs)
    trace = bool(int(os.environ.get("BASS_KT_TRACE", "0")))
    res = run_bass_kernel_spmd(nc, in_maps, list(range(NCORES)), trace=trace)
    kernel.last_results = res
    out = np.stack([np.asarray(res.results[k]["out"]) for k in range(NCORES)])
    return out.astype(np.float32)


# revision 32
# speedup vs baseline: 1.4927x; 1.0998x over previous
# kernel.py — Trainium2 Bass kernel for nn_ChannelAttentionBlock (v3)
#
# Computation (per reference):
#   h = relu(feature @ fc1_w.T + fc1_b)            [B,C,FF]
#   f = h @ fc2_w.T + fc2_b                        [B,C,HW]
#   T[b,n,m] = sum_c x[b,c,n] * f[b,c,m] * ls[m]   (ls = exp(min(logit_scale, log 100)))
#   P = softmax_n(T);  out[b,n,c] = sum_m P[n,m] x[b,c,m];  LayerNorm over c; -> [B,C,HW]
#
# Sharding (8 cores):
#   MLP tensor-parallel: fc1 split on hidden (each core 1152 of 9216 hidden),
#   fc2 split on input; ReduceScatter over batch so core k receives f[b=k].
#   Attention data-parallel: core k handles batch k entirely.
#
# Precision (PE streams 1 col/cyc for bf16/fp16, fp32r is 1 cyc when N>=256):
#   MLP1/MLP2: fp16 weights single-plane + fp16 hi/lo activations, 2 pairings
#     (fh.w + fl.w) — measured 7e-3 rel err vs the 2e-2 budget. Halves w1 DMA
#     (21.2MB/core) and cuts MLP PE work by a third vs bf16 3-way.
#   mm1 (T = f^T x, K=24): bf16 hi/lo K-stacked — full precision, free in K.
#   mm2 (out += xp^T E): fp32r on fp32 et (1cyc/row at N=512).
#   Softmax shift: per-m Cauchy-Schwarz bound -ls*xmax*||f[:,m]||+40.
#
# Schedule (v3): phase-separated for PE clock-gate hygiene. The PE drops to
# 1.2 GHz after a >3.4us idle and only re-promotes after ~3us of gapless
# execution, so work is organized in long wait-free streaks:
#   [w1 DMA || MLP1] -> [MLP2 all 4 groups back-to-back; RS(g) issued as each
#   group finishes, RS wait hidden under later groups] -> [attention: 32
#   m-block slots; 2048-wide exp on 2x[128,4,512] PSUM rotation (all 8 banks);
#   mm2 of the lagged pair writes into exp-drained sub-banks of the previous
#   tile; per-slot PE burst ~3.5us vs ACT 4.2us] -> LayerNorm.

import os
import numpy as np

B, C, HW, FF, P = 8, 6, 4096, 9216, 128
NCORES = 8
HS = FF // NCORES        # 1152
KT1 = FF // P            # 72 fc1 K tiles
CH1 = 3                  # fc1 K tiles per DMA chunk
NC1 = KT1 // CH1         # 24 fc1 chunks
JT = HS // P             # 9  fc2 K tiles
NT = HW // 512           # 8  512-wide n/o chunks
MBS = HW // P            # 32 m blocks
NGRP = 4                 # reduce-scatter groups (2 o-chunks each)
MB_PER_G = MBS // NGRP   # 8
EPS = 1e-5
SHIFT_HEADROOM = 40.0

_cache = {}


def _build_program():
    import concourse.bacc as bacc
    import concourse.bass as bass
    import concourse.tile as tile
    import concourse.mybir as mybir

    dt = mybir.dt.float32
    dtr = mybir.dt.float32r
    dtb = mybir.dt.bfloat16
    dth = mybir.dt.float16
    AF = mybir.ActivationFunctionType
    ALU = mybir.AluOpType
    AX = mybir.AxisListType

    nc = bacc.Bacc(
        "TRN2",
        target_bir_lowering=False,
        debug=False,
        enable_asserts=False,
        num_devices=NCORES,
    )

    # ---- external I/O ----
    featT_d = nc.dram_tensor("featT", [3, P, KT1 // 3, 2, 48], dth,
                             kind="ExternalInput").ap()
    w1_d = nc.dram_tensor("w1t", [NC1, P, CH1, HS], dth, kind="ExternalInput").ap()
    b1_d = nc.dram_tensor("b1", [1, HS], dt, kind="ExternalInput").ap()
    w2_d = nc.dram_tensor("w2t", [NGRP, P, JT, 1024], dth, kind="ExternalInput").ap()
    b2_d = nc.dram_tensor("b2", [1, HW], dt, kind="ExternalInput").ap()  # fc2_b/8
    xaug_d = nc.dram_tensor("xaug", [24, HW], dtb, kind="ExternalInput").ap()
    xtb_d = nc.dram_tensor("xtb", [P, MBS, C], dt, kind="ExternalInput").ap()
    ls_d = nc.dram_tensor("lsb", [P, MBS], dt, kind="ExternalInput").ap()
    srt_d = nc.dram_tensor("srtc", [P, 2], mybir.dt.int32,
                           kind="ExternalInput").ap()
    ones_d = nc.dram_tensor("ones1", [1, 48], dt, kind="ExternalInput").ap()
    id48_d = nc.dram_tensor("id48", [48, 48], dt, kind="ExternalInput").ap()
    blk_d = nc.dram_tensor("blk", [48, NT], dt, kind="ExternalInput").ap()
    blkT_d = nc.dram_tensor("blkT", [NT, 48], dt, kind="ExternalInput").ap()
    wb48_d = nc.dram_tensor("wb48", [48, 2], dt, kind="ExternalInput").ap()
    negx_d = nc.dram_tensor("negx", [P, 1], dt, kind="ExternalInput").ap()
    out_d = nc.dram_tensor("out", [C, HW], dt, kind="ExternalOutput").ap()

    with tile.TileContext(nc) as tc:
        # float32r APs carry full-fp32 bit patterns; the PE rounds at load.
        with nc.allow_low_precision(reason="fp32r/fp16/bf16 kernel dataflow"), \
             tc.tile_pool(name="const", bufs=1) as const, \
             tc.tile_pool(name="dram", bufs=1, space="DRAM") as dram:

            # ---- constants / small inputs ----
            xaug_sb = const.tile([24, HW], dtb, tag="xaug")
            nc.gpsimd.dma_start(out=xaug_sb[:], in_=xaug_d)
            xtb_sb = const.tile([P, MBS, C], dt, tag="xtb")
            nc.gpsimd.dma_start(out=xtb_sb[:], in_=xtb_d)
            ls_sb = const.tile([P, MBS], dt, tag="ls")
            nc.gpsimd.dma_start(out=ls_sb[:], in_=ls_d)
            ones_sb = const.tile([1, 48], dtr, tag="ones1")
            nc.gpsimd.dma_start(out=ones_sb[:], in_=ones_d.bitcast(dtr))
            id48_sb = const.tile([48, 48], dt, tag="id48")
            nc.gpsimd.dma_start(out=id48_sb[:], in_=id48_d)
            negx_sb = const.tile([P, 1], dt, tag="negx")
            nc.gpsimd.dma_start(out=negx_sb[:], in_=negx_d)
            srt_sb = const.tile([P, 2], mybir.dt.int32, tag="srtc")
            nc.gpsimd.dma_start(out=srt_sb[:], in_=srt_d)
            blk_sb = const.tile([48, NT], dtr, tag="blk")
            nc.gpsimd.dma_start(out=blk_sb[:], in_=blk_d.bitcast(dtr))
            blkT_sb = const.tile([NT, 48], dtr, tag="blkT")
            nc.gpsimd.dma_start(out=blkT_sb[:], in_=blkT_d.bitcast(dtr))
            wb48_sb = const.tile([48, 2], dt, tag="wb48")
            nc.gpsimd.dma_start(out=wb48_sb[:], in_=wb48_d)

            # h^T in fp16 hi/lo planes (filled after MLP1)
            hT_sb = const.tile([P, JT, 2, 48], dth, tag="hT")
            faug_sb = const.tile([24, HW], dtb, tag="faug")
            # per-m softmax shift (exp bias): -ls*xmax*||f[:,m]|| + 40
            shift_sb = const.tile([P, MBS], dt, tag="shift")
            zero128_sb = const.tile([P, 1], dt, tag="zero128")
            nc.vector.memset(zero128_sb[:], 0.0)
            ones6_sb = const.tile([C, 1], dtb, tag="ones6")
            nc.vector.memset(ones6_sb[:], 1.0)
            # attention output accumulators, one [6,512] tile per n-chunk
            O_nt = [const.tile([C, 512], dt, tag=f"O{nt}", name=f"O{nt}")
                    for nt in range(NT)]
            wz_sb = const.tile([1, 640], dt, tag="wz")
            nc.vector.memset(wz_sb[:], 0.0)
            O48_sb = const.tile([48, 512], dt, tag="O48")

            # DRAM bounce buffers for the reduce-scatter groups
            rs_in = [dram.tile([48, 2, 512], dt, tag=f"rsin{g}", name=f"rsin{g}")
                     for g in range(NGRP)]
            rs_out = [dram.tile([C, 2, 512], dt, tag=f"rsout{g}", name=f"rsout{g}")
                      for g in range(NGRP)]
            wrm_in = dram.tile([1, 8], dt, tag="wrmin", name="wrmin")
            wrm_out = dram.tile([1, 8], dt, tag="wrmout", name="wrmout")

            # ============ warmup: PE clock promote, exp tables, CC cores ============
            with tc.tile_pool(name="wrm", bufs=1, space="PSUM") as wrmp:
                wps = wrmp.tile([P, 512], dt, tag="wps")
                for _ in range(12):
                    nc.tensor.matmul(
                        wps[:],
                        lhsT=wz_sb[:, 0:P].bitcast(dtr),
                        rhs=wz_sb[:, P:P + 512].bitcast(dtr),
                        start=True,
                        stop=True,
                    )
                # load the exp table set now (Exp+Relu share it; the kernel
                # body uses no other table until the final LayerNorm)
                wex = wrmp.tile([1, 8], dt, tag="wex")
                nc.scalar.activation(wex[:], wz_sb[:, 0:8], AF.Exp,
                                     bias=zero128_sb[0:1, :])
            # init + warm the collective path (RDH state, cross-core skew)
            nc.gpsimd.dma_start(out=wrm_in[:], in_=wz_sb[:, 0:8])
            nc.gpsimd.collective_compute(
                "AllReduce",
                ALU.add,
                replica_groups=[list(range(NCORES))],
                ins=[wrm_in.opt()],
                outs=[wrm_out.opt()],
            )

            # ====== MLP1 (w2 DMAs interleaved) / MLP2 + RS / fprep(0) ======
            with tc.tile_pool(name="w2p", bufs=NGRP) as w2p, \
                 tc.tile_pool(name="m2c", bufs=1) as m2c, \
                 tc.tile_pool(name="fts", bufs=2) as ftsp, \
                 tc.tile_pool(name="fps", bufs=2, space="PSUM") as fps, \
                 tc.tile_pool(name="fpe", bufs=2) as fpep:

                def emit_fprep(g, get_ptile):
                    # runs once RS(g) lands: distribute f, build faug + shift.
                    # get_ptile() -> [P, 512] PSUM AP for the norm matmuls.
                    gsl = slice(g * 1024, (g + 1) * 1024)
                    fgrp = ftsp.tile([C, 1024], dt, tag="fgrp")
                    nc.sync.dma_start(out=fgrp[:], in_=rs_out[g][:])
                    # bf16 hi/lo split of f for mm1, rows [fh,fh,fl,fl]
                    fh = ftsp.tile([C, 1024], dtb, tag="fh")
                    nc.vector.tensor_copy(fh[:], fgrp[:])
                    fl = ftsp.tile([C, 1024], dtb, tag="fl")
                    nc.vector.tensor_sub(fl[:], fgrp[:], fh[:])
                    nc.sync.dma_start(out=faug_sb[0:C, gsl], in_=fh[:])
                    nc.sync.dma_start(out=faug_sb[C:2 * C, gsl], in_=fh[:])
                    nc.sync.dma_start(out=faug_sb[2 * C:3 * C, gsl], in_=fl[:])
                    nc.sync.dma_start(out=faug_sb[3 * C:24, gsl], in_=fl[:])
                    # shift[m] = -ls[m]*xmax*||f[:,m]||_2 + 40; column sq-norms
                    # via tiny matmuls into a borrowed PSUM tile, then sqrt on
                    # DVE via the bitcast seed (+-3.5%, harmless in a bound
                    # with ~47 e-folds of margin). No ACT => no table reloads.
                    fsq = ftsp.tile([C, 1024], dtb, tag="fsq")
                    nc.vector.tensor_mul(fsq[:], fgrp[:], fgrp[:])
                    pt = get_ptile()
                    for chn in range(NT):
                        nc.tensor.matmul(
                            pt[:, chn:chn + 1],
                            lhsT=fsq[:, chn * P:(chn + 1) * P],
                            rhs=ones6_sb[:],
                            start=True,
                            stop=True,
                        )
                    # y = bitcast(i >> 1) = sqrt(S) * 2^-63.5 * (1..1.061);
                    # the 2^63.5 rescale is folded into negx on the host
                    fns = ftsp.tile([P, NT], mybir.dt.int32, tag="fns")
                    nc.vector.tensor_scalar(
                        out=fns[:], in0=pt[:, 0:NT].bitcast(mybir.dt.int32),
                        scalar1=srt_sb[:, 0:1], scalar2=0.0,
                        op0=ALU.arith_shift_right, op1=ALU.bypass,
                    )
                    fnl = ftsp.tile([P, NT], dt, tag="fnl")
                    nc.vector.tensor_mul(
                        fnl[:], fns[:].bitcast(dt),
                        ls_sb[:, NT * g:NT * (g + 1)])
                    nc.vector.tensor_scalar(
                        out=shift_sb[:, NT * g:NT * (g + 1)], in0=fnl[:],
                        scalar1=negx_sb[:], scalar2=SHIFT_HEADROOM,
                        op0=ALU.mult, op1=ALU.add,
                    )

                b2_sb = m2c.tile([1, HW], dtr, tag="b2")
                nc.gpsimd.dma_start(out=b2_sb[:], in_=b2_d.bitcast(dtr))
                w2t = {}

                def fetch_w2(g, token=None):
                    # `token`: 1-elem copy into the tile first, so the DMA
                    # inherits a dependency and the scheduler cannot hoist it
                    # to t=0 where it would starve the MLP1 weight stream
                    w2t[g] = w2p.tile([P, JT, 1024], dth, tag="w2t",
                                      name=f"w2t{g}")
                    if token is not None:
                        nc.vector.tensor_copy(w2t[g][0:1, 0, 0:1], token)
                    # gpsimd ring: idle during MLP1, so a token-gated fetch
                    # can never head-of-line-block the w1 chunk stream
                    nc.gpsimd.dma_start(out=w2t[g][:], in_=w2_d[g])



                # ---- MLP1: h = relu(feat @ w1 + b1), fp16 2-way ----
                with tc.tile_pool(name="w1p", bufs=4) as w1p, \
                     tc.tile_pool(name="m1c", bufs=1) as m1c, \
                     tc.tile_pool(name="ps1", bufs=1, space="PSUM") as ps1, \
                     tc.tile_pool(name="pst", bufs=2, space="PSUM") as pst:
                    featT_sb = m1c.tile([P, KT1, 2, 48], dth, tag="featT")
                    for pc in range(3):  # split: piece 0 unblocks MLP1 early
                        nc.gpsimd.dma_start(
                            out=featT_sb[:, 24 * pc:24 * (pc + 1), :, :],
                            in_=featT_d[pc])
                    b1_sb = m1c.tile([1, HS], dtr, tag="b1")
                    nc.gpsimd.dma_start(out=b1_sb[:], in_=b1_d.bitcast(dtr))
                    h_sb = m1c.tile([48, HS], dt, tag="h")
                    hp = ps1.tile([48, 3, 512], dt, tag="hp")
                    for ch in range(NC1):
                        w1t = w1p.tile([P, CH1, HS], dth, tag="w1t")
                        nc.sync.dma_start(out=w1t[:], in_=w1_d[ch])
                        if ch == 15:
                            # w2 g0 starts once chunk 15 has landed: early
                            # enough to be resident for MLP2, late enough not
                            # to starve the w1 stream
                            fetch_w2(0, token=w1t[0:1, 0, 0:1])
                        for s in range(CH1):
                            kk = CH1 * ch + s
                            for hl in range(2):
                                lhs = featT_sb[:, kk, hl, :]
                                for j in range(3):
                                    nc.tensor.matmul(
                                        hp[:, j, 0:384],
                                        lhsT=lhs,
                                        rhs=w1t[:, s, j * 384:(j + 1) * 384],
                                        start=(kk == 0 and hl == 0),
                                        stop=False,
                                    )
                    for j in range(3):  # bias via K=1 ones row (fp32r)
                        nc.tensor.matmul(
                            hp[:, j, 0:384],
                            lhsT=ones_sb[:],
                            rhs=b1_sb[:, j * 384:(j + 1) * 384],
                            start=False,
                            stop=True,
                        )
                    for j in range(3):  # relu eviction on ACT
                        nc.scalar.activation(
                            h_sb[:, j * 384:(j + 1) * 384],
                            hp[:, j, 0:384],
                            AF.Relu,
                        )
                    # transpose h -> hT (9 PE transposes), split fp16 hi/lo
                    for t in range(JT):
                        tp = pst.tile([P, 48], dt, tag="tp")
                        nc.tensor.transpose(
                            tp[:], h_sb[:, t * P:(t + 1) * P], id48_sb[:]
                        )
                        nc.vector.tensor_copy(hT_sb[:, t, 0, :], tp[:])
                        nc.vector.tensor_sub(hT_sb[:, t, 1, :], tp[:],
                                             hT_sb[:, t, 0, :])

                # ---- MLP2 groups + RS; fprep(0) tucked after group 1 so its
                # faug/shift are ready the moment the PE finishes group 3 ----
                def emit_mlp2_group(g):
                    for occ in range(2):
                        oc = 2 * g + occ
                        fp = fps.tile([P, 512], dt, tag="fp")
                        for jj in range(JT):
                            for hl in range(2):
                                nc.tensor.matmul(
                                    fp[0:48, :],
                                    lhsT=hT_sb[:, jj, hl, :],
                                    rhs=w2t[g][:, jj, occ * 512:(occ + 1) * 512],
                                    start=(jj == 0 and hl == 0),
                                    stop=False,
                                )
                        nc.tensor.matmul(  # + fc2_b/8 (summed to fc2_b by RS)
                            fp[0:48, :],
                            lhsT=ones_sb[:],
                            rhs=b2_sb[:, oc * 512:(oc + 1) * 512],
                            start=False,
                            stop=True,
                        )
                        fpe = fpep.tile([48, 512], dt, tag="fpe")
                        nc.vector.tensor_copy(fpe[:], fp[0:48, :])
                        nc.sync.dma_start(out=rs_in[g][:, occ, :], in_=fpe[:])
                        if occ == 1:
                            nc.gpsimd.collective_compute(
                                "ReduceScatter",
                                ALU.add,
                                replica_groups=[list(range(NCORES))],
                                ins=[rs_in[g].opt()],
                                outs=[rs_out[g].opt()],
                            )

                fetch_w2(1, token=hT_sb[0:1, JT - 1, 1, 0:1])
                emit_mlp2_group(0)
                fetch_w2(2)       # reuses g0's buffer once its reads retire
                emit_mlp2_group(1)
                fetch_w2(3)
                emit_mlp2_group(2)
                emit_mlp2_group(3)
                # fprep(0) after all MLP2 matmuls: its tiny norm-matmuls gate
                # on RS(0), and nothing else may sit behind them in PE order
                emit_fprep(0, lambda: fps.tile([P, 512], dt, tag="fp",
                                               name="fprep0ps")[:])

            # ===================== attention =====================
            with tc.tile_pool(name="tpp", bufs=2, space="PSUM") as tpp, \
                 tc.tile_pool(name="cps", bufs=2, space="PSUM") as cpsp, \
                 tc.tile_pool(name="fts2", bufs=2) as ftsp2, \
                 tc.tile_pool(name="etp", bufs=5) as etp, \
                 tc.tile_pool(name="xpp", bufs=5) as xpp, \
                 tc.tile_pool(name="accp", bufs=4) as accp:

                first_flush = [True] * NT
                ets = {}        # mb -> (et tile, xp tile)
                ftsp = ftsp2  # fprep called below uses the attention pool

                def emit_mm2_chunk(pmb, ntc):
                    # one n-chunk of pair (pmb, pmb+1): 2 accumulating MMs
                    # into a transient cp bank, then DVE-accumulate into O
                    cp = cpsp.tile([C, 512], dt, tag="cp")
                    for q in range(2):
                        et_q, xp_q = ets[pmb + q]
                        nc.tensor.matmul(
                            cp[:],
                            lhsT=xp_q[:].bitcast(dtr),
                            rhs=et_q[:, ntc, :].bitcast(dtr),
                            start=(q == 0),
                            stop=(q == 1),
                        )
                    if first_flush[ntc]:
                        nc.vector.tensor_copy(O_nt[ntc][:].bitcast(dtr), cp[:])
                        first_flush[ntc] = False
                    else:
                        nc.vector.tensor_add(
                            O_nt[ntc][:].bitcast(dtr), O_nt[ntc][:], cp[:])
                    if ntc == NT - 1:
                        del ets[pmb]
                        del ets[pmb + 1]

                # n-chunks per act chunk: 3+3+2, mm2 chunks interleaved 2+1+1
                TCH = ((0, 3, 2), (3, 3, 1), (6, 2, 1))

                def emit_slot(mb):
                    # one m-block: mm1/exp over 1536/1536/1024 PSUM chunks;
                    # mm2 of the pair lagged 2 m-blocks goes to the cp pool so
                    # it never waits on an in-flight exp read
                    et = etp.tile([P, NT, 512], dt, tag="et")
                    acc = accp.tile([P, 3], dt, tag="acc")
                    lhs = faug_sb[:, mb * P:(mb + 1) * P]
                    mm2c = (mb % 2) * 4
                    for tch, (n0, nw, nmm2) in enumerate(TCH):
                        tps = tpp.tile([P, 3, 512], dt, tag="tps")
                        for i in range(nw):
                            nt_ = n0 + i
                            nc.tensor.matmul(
                                tps[:, i, :],
                                lhsT=lhs,
                                rhs=xaug_sb[:, nt_ * 512:(nt_ + 1) * 512],
                                start=True,
                                stop=True,
                            )
                        nc.scalar.activation(
                            et[:, n0:n0 + nw, :].bitcast(dtr),
                            tps[:, 0:nw, :],
                            AF.Exp,
                            scale=ls_sb[:, mb:mb + 1],
                            bias=shift_sb[:, mb:mb + 1],
                            accum_out=acc[:, tch:tch + 1],
                        )
                        if mb >= 2:
                            pmb = (mb // 2 - 1) * 2
                            for _ in range(nmm2):
                                emit_mm2_chunk(pmb, mm2c)
                                mm2c += 1
                    cs = accp.tile([P, 1], dt, tag="cs")
                    nc.vector.reduce_sum(cs[:], acc[:], AX.X)
                    rc = accp.tile([P, 1], dt, tag="rc")
                    nc.vector.reciprocal(rc[:], cs[:])
                    xp = xpp.tile([P, C], dt, tag="xp")
                    nc.vector.tensor_scalar_mul(
                        xp[:].bitcast(dtr), xtb_sb[:, mb, :], rc[:])
                    ets[mb] = (et, xp)

                def tpp_ptile():
                    t = tpp.tile([P, 3, 512], dt, tag="tps", name="tpsx")
                    return t[:, 0, :]

                # fprep(0) already ran during MLP2; prep group g+1 one group
                # early, two slots in (so its RS-gated matmuls sit behind
                # already-runnable slot work in the PE stream)
                for g in range(NGRP):
                    for mb in range(g * MB_PER_G, (g + 1) * MB_PER_G):
                        emit_slot(mb)
                        if mb == g * MB_PER_G + 1 and g + 1 < NGRP:
                            emit_fprep(g + 1, tpp_ptile)
                # drain: last pair's mm2 chunks
                for ntc in range(NT):
                    emit_mm2_chunk(MBS - 2, ntc)

                # stack the 8 [6,512] chunks into [48,512] (partition-offset
                # SBUF->SBUF DMAs)
                for nt_ in range(NT):
                    nc.sync.dma_start(
                        out=O48_sb[C * nt_:C * nt_ + C, :].bitcast(dtr),
                        in_=O_nt[nt_][:].bitcast(dtr))

            # ===================== LayerNorm over c + output =====================
            with tc.tile_pool(name="lnps", bufs=2, space="PSUM") as lnps, \
                 tc.tile_pool(name="lnrp", bufs=2, space="PSUM") as lnrp, \
                 tc.tile_pool(name="lnsb", bufs=1) as lnsb:
                eps_sb = lnsb.tile([NT, 1], dt, tag="eps")
                nc.vector.memset(eps_sb[:], EPS)
                O2_sb = lnsb.tile([48, 512], dt, tag="O2")
                nc.vector.tensor_mul(O2_sb[:].bitcast(dtr), O48_sb[:], O48_sb[:])
                s_ps = lnps.tile([NT, 512], dt, tag="sps")
                nc.tensor.matmul(
                    s_ps[:], lhsT=blk_sb[:], rhs=O48_sb[:].bitcast(dtr),
                    start=True, stop=True,
                )
                s2_ps = lnps.tile([NT, 512], dt, tag="s2ps")
                nc.tensor.matmul(
                    s2_ps[:], lhsT=blk_sb[:], rhs=O2_sb[:].bitcast(dtr),
                    start=True, stop=True,
                )
                mean_sb = lnsb.tile([NT, 512], dt, tag="mean")
                nc.vector.tensor_scalar_mul(
                    mean_sb[:].bitcast(dtr), s_ps[:], 1.0 / C)
                ms_sb = lnsb.tile([NT, 512], dt, tag="ms")
                nc.vector.tensor_mul(ms_sb[:], mean_sb[:], mean_sb[:])
                var_sb = lnsb.tile([NT, 512], dt, tag="var")
                nc.vector.tensor_scalar_mul(var_sb[:], s2_ps[:], 1.0 / C)
                nc.vector.tensor_sub(var_sb[:], var_sb[:], ms_sb[:])
                # 1/sqrt(var+eps) via ln/exp (same resident table set)
                vln_sb = lnsb.tile([NT, 512], dt, tag="vln")
                nc.scalar.activation(vln_sb[:], var_sb[:], AF.Ln, bias=eps_sb[:])
                rstd_sb = lnsb.tile([NT, 512], dt, tag="rstd")
                nc.scalar.activation(rstd_sb[:].bitcast(dtr), vln_sb[:], AF.Exp,
                                     scale=-0.5, bias=zero128_sb[0:NT, :])
                mrep = lnrp.tile([48, 512], dt, tag="mrep")
                nc.tensor.matmul(
                    mrep[:], lhsT=blkT_sb[:], rhs=mean_sb[:].bitcast(dtr),
                    start=True, stop=True,
                )
                rrep = lnrp.tile([48, 512], dt, tag="rrep")
                nc.tensor.matmul(
                    rrep[:], lhsT=blkT_sb[:], rhs=rstd_sb[:].bitcast(dtr),
                    start=True, stop=True,
                )
                on_sb = lnsb.tile([48, 512], dt, tag="on")
                nc.vector.tensor_sub(on_sb[:], O48_sb[:], mrep[:])
                nc.vector.tensor_mul(on_sb[:], on_sb[:], rrep[:])
                nc.vector.tensor_scalar(
                    out=on_sb[:], in0=on_sb[:],
                    scalar1=wb48_sb[:, 0:1], scalar2=wb48_sb[:, 1:2],
                    op0=ALU.mult, op1=ALU.add,
                )
                for nt_ in range(NT):
                    nc.sync.dma_start(
                        out=out_d[:, nt_ * 512:(nt_ + 1) * 512],
                        in_=on_sb[C * nt_:C * nt_ + C, :],
                    )

    nc.compile()
    return nc


def _host_prep(inputs):
    import ml_dtypes
    bf16 = ml_dtypes.bfloat16

    x = np.asarray(inputs["x"], np.float32)
    feature = np.asarray(inputs["feature"], np.float32)
    fc1_w = np.asarray(inputs["fc1_w"], np.float32)
    fc1_b = np.asarray(inputs["fc1_b"], np.float32)
    fc2_w = np.asarray(inputs["fc2_w"], np.float32)
    fc2_b = np.asarray(inputs["fc2_b"], np.float32)
    logit_scale = np.asarray(inputs["logit_scale"], np.float32)
    norm_w = np.asarray(inputs["norm_w"], np.float32)
    norm_b = np.asarray(inputs["norm_b"], np.float32)

    def split_hl(a, dtyp):
        hi = a.astype(dtyp)
        lo = (a - hi.astype(np.float32)).astype(dtyp)
        return hi, lo

    w1T = np.ascontiguousarray(fc1_w.T)                      # [f, h]
    w2T = np.ascontiguousarray(fc2_w.T)                      # [h, o]
    featT = np.ascontiguousarray(feature.reshape(B * C, FF).T)   # [f, bc]
    fth, ftl = split_hl(featT, np.float16)
    # [3, 128, 24, 2, 48] — three k-range pieces for split DMA
    featT_b = np.ascontiguousarray(
        np.stack([fth, ftl], axis=1).reshape(KT1, P, 2, B * C)
        .transpose(1, 0, 2, 3).reshape(P, 3, KT1 // 3, 2, B * C)
        .transpose(1, 0, 2, 3, 4))
    ls = np.exp(np.minimum(logit_scale.reshape(HW), np.log(np.float32(100.0))))
    ls_b = np.ascontiguousarray(ls.reshape(MBS, P).T).astype(np.float32)
    ones1 = np.ones((1, B * C), np.float32)
    id48 = np.eye(48, dtype=np.float32)
    blk = np.zeros((48, NT), np.float32)
    blk[np.arange(48), np.arange(48) // C] = 1.0
    blkT = np.ascontiguousarray(blk.T)
    wb48 = np.ascontiguousarray(
        np.stack([np.tile(norm_w, NT), np.tile(norm_b, NT)], axis=1))
    b2 = (fc2_b / NCORES).reshape(1, HW).astype(np.float32)

    in_maps = []
    for k in range(NCORES):
        w1k = w1T[:, k * HS:(k + 1) * HS].astype(np.float16)   # [9216, 1152]
        # [24, 128, 3, 1152]
        w1s = np.ascontiguousarray(
            w1k.reshape(NC1, CH1, P, HS).transpose(0, 2, 1, 3))
        b1k = np.ascontiguousarray(fc1_b[k * HS:(k + 1) * HS]).reshape(1, HS)
        w2k = w2T[k * HS:(k + 1) * HS, :].astype(np.float16)   # [1152, 4096]
        # [4, 128, 9, 1024]
        w2s = np.ascontiguousarray(
            w2k.reshape(JT, P, NGRP, 1024).transpose(2, 1, 0, 3))
        xh, xl = split_hl(x[k], bf16)                          # [6, 4096]
        xaug = np.concatenate([xh, xl, xh, xl], axis=0)        # [24, 4096]
        xtbk = np.ascontiguousarray(x[k].T.reshape(MBS, P, C).transpose(1, 0, 2))
        xmax_k = np.linalg.norm(x[k], axis=0).max()
        # 2^63.5 rescale of the bitcast-sqrt seed folded in
        negx = np.full((P, 1), -xmax_k * 2.0**63.5, np.float32)
        srt = np.ascontiguousarray(
            np.broadcast_to(np.array([[1, 0x1FBD1DF5]], np.int32), (P, 2)))
        in_maps.append({
            "featT": featT_b, "w1t": w1s, "b1": b1k, "w2t": w2s, "b2": b2,
            "xaug": xaug, "xtb": xtbk, "lsb": ls_b, "ones1": ones1,
            "id48": id48, "blk": blk, "blkT": blkT, "wb48": wb48,
            "negx": negx, "srtc": srt,
        })
    return in_maps


def _install_ntff_shim():
    # The agent image's `antenv` lacks `axon_hooks`, which bass_utils needs
    # for trace=True under axon. Fabricate the registry module and install
    # the ctypes-based NTFF hook against libaxon_pjrt.so.
    import sys
    import types
    import ctypes
    import contextlib

    try:
        import antenv.axon_hooks  # noqa: F401
        return
    except ImportError:
        pass
    if "antenv.axon_hooks" in sys.modules:
        return
    mod = types.ModuleType("antenv.axon_hooks")
    _h = [None]
    mod.set_axon_ntff_profile_hook = lambda h: _h.__setitem__(0, h)
    mod.get_axon_ntff_profile_hook = lambda: _h[0]
    sys.modules["antenv.axon_hooks"] = mod

    so_path = "/opt/axon/libaxon_pjrt.so"
    if not os.path.exists(so_path):
        return
    lib = ctypes.CDLL(so_path)
    if not hasattr(lib, "axon_start_nrt_profile"):
        return
    lib.axon_start_nrt_profile.argtypes = [
        ctypes.POINTER(ctypes.c_int64), ctypes.c_size_t]
    lib.axon_start_nrt_profile.restype = ctypes.c_int64
    lib.axon_stop_nrt_profile.argtypes = [ctypes.c_char_p]
    lib.axon_stop_nrt_profile.restype = ctypes.c_int64

    @contextlib.contextmanager
    def _hook(output_dir, device_ids):
        import jax
        jax.devices()
        if device_ids:
            ids = (ctypes.c_int64 * len(device_ids))(*device_ids)
            rc = lib.axon_start_nrt_profile(ids, len(device_ids))
        else:
            rc = lib.axon_start_nrt_profile(None, 0)
        if rc != 0:
            raise RuntimeError(f"axon_start_nrt_profile rc={rc}")
        try:
            yield
        finally:
            n = lib.axon_stop_nrt_profile(str(output_dir).encode())
            print(f"ntff profile: {n} file(s) written to {output_dir}")

    mod.set_axon_ntff_profile_hook(_hook)


def kernel(**inputs):
    from concourse.bass_utils import run_bass_kernel_spmd

    if bool(int(os.environ.get("BASS_KT_TRACE", "0"))):
        _install_ntff_shim()

    if "nc" not in _cache:
        _cache["nc"] = _build_program()
    nc = _cache["nc"]

    in_maps = _host_prep(inputs)
    trace = bool(int(os.environ.get("BASS_KT_TRACE", "0")))
    res = run_bass_kernel_spmd(nc, in_maps, list(range(NCORES)), trace=trace)
    kernel.last_results = res
    out = np.stack([np.asarray(res.results[k]["out"]) for k in range(NCORES)])
    return out.astype(np.float32)
